# revision 14
# baseline (speedup 1.0000x reference)
"""Trainium2 Bass kernel for the CPC loss (nn_CPC_292057776614) — v2.

Data-parallel over the prediction axis (8 cores, 1120 preds each, padded
to 1152 = 9 tiles of 128). The ctx-row gather and the positive-candidate
row gather are done ON HOST (indices are host-known), pre-transposed
into the [d_low, k, p] SBUF layout the PE needs — so the device runs no
indirect DMAs and no xbar transposes (both serialized badly against the
bulk weight/encoding streams in v1).

Per core:
  - stage A: predT[dout, p] = W_s^T-contraction of ctxT, 10 PSUM
    k-accumulated matmuls per (s, m-chunk); bias applied during the ACT
    PSUM->SBUF evacuation (bf16 cast).
  - per tile t (128 preds):
    - dots0 exactly on PE: 10 matmuls predT_t x c0T_t -> PSUM [128,128];
      diagonal extracted with one fused DVE scalar_tensor_tensor
      (identity mask multiply + row-sum accumulator).
    - stage B all-pairs scores vs all 3136 encodings: k-outer matmuls
      (lhsT stationary per k, 3 chunk-groups x {3,2,2} PSUM banks); each
      chunk evacuated by a DVE tensor_add with the additive candidate
      mask C (= ln(multiplicity) on candidate slots / -1e30 elsewhere).
    - negmax via one negated DVE row-max; sumexp via ACT exp with
      accumulate (bias = negmax).
  - loss_p = ln(sumexp) - negmax - dots0 ; corr_p = dots0 >= -negmax;
    masked by the valid mask, reduced via a ones-vector matmul.

Host sums the 8 per-core [loss_sum, correct_sum] pairs / 8960.

DMA schedule: one sync-ring (SP HWDGE) FIFO in need order
  W0, ctxT(3 chunks), encA, cm0, W1, cm1, encB, cm2, W2, cm3, W3, cm4,
  W4, cm5..cm8
plus the small consts and the per-tile c0T prefetches on the scalar
(ACT HWDGE) ring. PE emission interleaves stage-A steps with stage-B
tiles so the weight stream stays ahead of compute.
"""

import numpy as np
import ml_dtypes

import concourse.bass as bass
import concourse.mybir as mybir
import concourse.tile as tile
from concourse import bacc
from concourse.bass_utils import run_bass_kernel_spmd
from concourse.masks import make_identity

BF16 = mybir.dt.bfloat16
F32 = mybir.dt.float32
I32 = mybir.dt.int32

# Problem constants (hardcoded; kernel.py must be self-contained).
B, G, D, S, NEG = 64, 7, 1280, 5, 16
CELLS = G * G            # 49
R = B * CELLS            # 3136 rows in ctx/enc
K17 = NEG + 1            # 17 candidates per prediction
STEP_LENS = [B * (G - 1 - s) * G for s in range(S)]     # [2688,2240,1792,1344,896]
P_TOTAL = sum(STEP_LENS)                                # 8960
N_CORES = 8
L = [sl // N_CORES for sl in STEP_LENS]                 # [336,280,224,168,112]
PC = sum(L)                                             # 1120 per core
NT = 9                                                  # p-tiles of 128
PP = NT * 128                                           # 1152 padded
PO = [sum(L[:s]) for s in range(S)]                     # per-core step offsets
KD = D // 128                                           # 10 k-tiles
ECH = 448                                               # e-chunk width (448*7=3136)
NE = R // ECH                                           # 7 chunks
GROUPS = [(0, 1, 2), (3, 4), (5, 6)]                    # stage-B chunk groups
CW2 = PP - L[0] - L[1]                                  # 536: s2+s3+s4+pad ctx cols
NEGINF = -1.0e30
NWARM = 200
GT = (1, 3)                                             # gather-path tiles
APT = tuple(t for t in range(NT) if t not in GT)        # all-pairs tiles                                             # HAM warmup matmuls

_CACHE = {}

DEBUG = bool(int(__import__("os").environ.get("BASS_CPC_DEBUG", "0")))


def _build():
    if "nc" in _CACHE:
        return _CACHE["nc"]

    nc = bacc.Bacc("TRN2", target_bir_lowering=False, debug=False)

    ctxA_d = nc.dram_tensor("ctxA", [128, KD * L[0]], BF16, kind="ExternalInput")
    ctxB_d = nc.dram_tensor("ctxB", [128, KD * L[1]], BF16, kind="ExternalInput")
    ctxC_d = nc.dram_tensor("ctxC", [128, KD * CW2], BF16, kind="ExternalInput")
    c0T_d = nc.dram_tensor("c0T", [NT, 128, KD * 128], BF16, kind="ExternalInput")
    encT_d = nc.dram_tensor("encT", [D, R], BF16, kind="ExternalInput")
    WT_d = nc.dram_tensor("WT", [S, 128, KD * D], BF16, kind="ExternalInput")
    bias_d = nc.dram_tensor("biasT", [128, S * KD], F32, kind="ExternalInput")
    vmask_d = nc.dram_tensor("vmask", [128, NT], F32, kind="ExternalInput")
    cmask_d = nc.dram_tensor("cmask", [PP, R], BF16, kind="ExternalInput")
    cand_d = {
        t: nc.dram_tensor(f"cand{t}", [K17, 128, D], BF16, kind="ExternalInput")
        for t in GT
    }
    out_d = nc.dram_tensor("out", [1, 2], F32, kind="ExternalOutput")
    if DEBUG:
        predT_dbg = nc.dram_tensor("predT_dbg", [128, KD, PP], BF16, kind="ExternalOutput")
        cols_dbg = nc.dram_tensor("cols_dbg", [128, 4 * NT], F32, kind="ExternalOutput")

    with tile.TileContext(nc) as tc:
        with (
            tc.tile_pool(name="const", bufs=1) as const,
            tc.tile_pool(name="c0tp", bufs=2) as c0tp,
            tc.tile_pool(name="wpool", bufs=2) as wpool,
            tc.tile_pool(name="cmp", bufs=2) as cmp,
            tc.tile_pool(name="mkp", bufs=1) as mkp,
            tc.tile_pool(name="candp", bufs=2) as candp,
            tc.tile_pool(name="etp", bufs=1) as etp,
            tc.tile_pool(name="psA", bufs=2, space="PSUM") as psA,
            tc.tile_pool(name="psB", bufs=5, space="PSUM") as psB,
            tc.tile_pool(name="psD", bufs=1, space="PSUM") as psD,
        ):
            # ---- small consts (scalar HWDGE ring, first) ----
            bias_sb = const.tile([128, S * KD], F32)
            nc.scalar.dma_start(out=bias_sb[:], in_=bias_d.ap())
            vmask_sb = const.tile([128, NT], F32)
            nc.scalar.dma_start(out=vmask_sb[:], in_=vmask_d.ap())

            identB = const.tile([128, 128], BF16)
            make_identity(nc, identB[:])
            ones = const.tile([128, 1], F32)
            nc.vector.memset(ones[:], 1.0)

            # HAM warmup: keep the PE busy during the initial load window so
            # the clock gate opens before real compute starts. Accumulating
            # chain (DCE-safe); result kept alive by a copy nobody uses.
            wps = psD.tile([128, 128], F32, tag="d")
            for i in range(NWARM):
                nc.tensor.matmul(
                    wps[:], lhsT=identB[:], rhs=identB[:],
                    start=(i == 0), stop=(i == NWARM - 1),
                )
            warmkeep = const.tile([128, 1], F32)
            nc.vector.tensor_copy(warmkeep[:], wps[:, 0:1])

            encT_sb = const.tile([128, KD, R], BF16)
            ctxA_sb = const.tile([128, KD, L[0]], BF16)
            ctxB_sb = const.tile([128, KD, L[1]], BF16)
            ctxC_sb = const.tile([128, KD, CW2], BF16)
            predT_sb = const.tile([128, KD, PP], BF16)
            nc.vector.memset(predT_sb[:, :, PC:PP], 0.0)

            negmax = const.tile([128, NT], F32)
            nmx = const.tile([128, NT * 8], F32)
            dots0 = const.tile([128, NT], F32)
            sume = const.tile([128, NT], F32)
            lnS = const.tile([128, NT], F32)
            junkw = const.tile([128, D], BF16)
            res = const.tile([128, 2 * NT], F32)
            dg = const.tile([128, len(GT) * K17], F32)
            et17 = const.tile([128, len(GT) * K17], BF16)
            prednat = const.tile([128, len(GT) * D], BF16)

            # ---- c0T per-tile prefetch (scalar ring) ----
            c0ts = {}

            def emit_c0t(t):
                ct = c0tp.tile([128, KD, 128], BF16, tag="c0t")
                nc.scalar.dma_start(
                    out=ct[:],
                    in_=c0T_d.ap()[t].rearrange("p (k j) -> p k j", k=KD),
                )
                c0ts[t] = ct

            emit_c0t(APT[0])
            emit_c0t(APT[1])

            # ---- big streams (sync HWDGE ring) in need order ----
            wtiles = {}

            def emit_w(s):
                w = wpool.tile([128, KD, D], BF16, tag="w")
                wsrc = WT_d.ap()[s].rearrange("p (k j) -> p k j", k=KD)
                nc.sync.dma_start(out=w[:], in_=wsrc)
                wtiles[s] = w

            cmts = {}

            def emit_cm(t):
                cm = cmp.tile([128, R], BF16, tag="cm")
                nc.sync.dma_start(
                    out=cm[:], in_=cmask_d.ap()[t * 128:(t + 1) * 128, :]
                )
                cmts[t] = cm

            encsrc = encT_d.ap().rearrange("(k p) e -> p k e", p=128)
            nc.sync.dma_start(
                out=ctxA_sb[:], in_=ctxA_d.ap().rearrange("p (k j) -> p k j", k=KD)
            )
            emit_w(0)
            nc.sync.dma_start(
                out=ctxB_sb[:], in_=ctxB_d.ap().rearrange("p (k j) -> p k j", k=KD)
            )
            nc.sync.dma_start(
                out=ctxC_sb[:], in_=ctxC_d.ap().rearrange("p (k j) -> p k j", k=KD)
            )
            nc.sync.dma_start(out=encT_sb[:, :, 0:4 * ECH], in_=encsrc[:, :, 0:4 * ECH])
            emit_cm(0)
            emit_w(1)
            nc.sync.dma_start(out=encT_sb[:, :, 4 * ECH:R], in_=encsrc[:, :, 4 * ECH:R])
            emit_cm(2)
            emit_w(2)
            emit_w(3)
            emit_cm(4)
            emit_w(4)
            for t in range(5, NT):
                emit_cm(t)

            # ---- compute emission: interleave A-steps, diag, B-tiles ----
            CTX_OF = {0: None, 1: None, 2: None, 3: None, 4: None}

            def _ctx_rhs(s, k):
                if s == 0:
                    return ctxA_sb[:, k, :]
                if s == 1:
                    return ctxB_sb[:, k, :]
                off = PO[s] - PO[2]
                return ctxC_sb[:, k, off:off + L[s]]

            def stage_a(s):
                w = wtiles[s]
                lo, ln = PO[s], L[s]
                for m in range(KD):
                    pa = psA.tile([128, ECH], F32, tag="a")
                    for k in range(KD):
                        nc.tensor.matmul(
                            pa[:, :ln],
                            lhsT=w[:, k, m * 128:(m + 1) * 128],
                            rhs=_ctx_rhs(s, k),
                            start=(k == 0),
                            stop=(k == KD - 1),
                        )
                    nc.scalar.activation(
                        predT_sb[:, m, lo:lo + ln],
                        pa[:, :ln],
                        mybir.ActivationFunctionType.Identity,
                        bias=bias_sb[:, s * KD + m:s * KD + m + 1],
                        scale=1.0,
                    )
                    drain(1)

            def diag(t):
                ct = c0ts[t]
                i = APT.index(t)
                if i + 2 < len(APT):
                    emit_c0t(APT[i + 2])
                rows = slice(t * 128, (t + 1) * 128)
                pd = psD.tile([128, 128], F32, tag="d")
                for k in range(KD):
                    nc.tensor.matmul(
                        pd[:],
                        lhsT=predT_sb[:, k, rows],
                        rhs=ct[:, k, :],
                        start=(k == 0),
                        stop=(k == KD - 1),
                    )
                nc.vector.scalar_tensor_tensor(
                    out=junkw[:, 0:128], in0=pd[:], scalar=0.0, in1=identB[:],
                    op0=mybir.AluOpType.add, op1=mybir.AluOpType.mult,
                    accum_out=dots0[:, t:t + 1],
                )

            pending = []

            def drain(n):
                for _ in range(min(n, len(pending))):
                    pending.pop(0)()

            def gather_tile(t):
                gi = GT.index(t)
                rows = slice(t * 128, (t + 1) * 128)
                pcols = slice(gi * D, (gi + 1) * D)
                # transpose predT tile -> natural [p, d] via PE (psA: 2 banks)
                for k in range(KD):
                    pt = psA.tile([128, 128], BF16, tag="a")
                    nc.tensor.transpose(
                        pt[:], predT_sb[:, k, rows], identB[:]
                    )
                    nc.vector.tensor_copy(
                        prednat[:, gi * D + k * 128:gi * D + (k + 1) * 128], pt[:]
                    )
                # queue the 17 fused dot ops + stats (drained between evacs);
                # cand DMAs dispatched one slot ahead of their dot op
                state = {"dma": 0, "tiles": {}}

                def _dispatch():
                    s = state["dma"]
                    if s < K17:
                        cs = candp.tile([128, D], BF16, tag="cand")
                        nc.scalar.dma_start(out=cs[:], in_=cand_d[t].ap()[s])
                        state["tiles"][s] = cs
                        state["dma"] = s + 1

                def mk_dot(slot):
                    def emit():
                        if slot == 0:
                            _dispatch()
                        _dispatch()
                        cs = state["tiles"].pop(slot)
                        nc.vector.scalar_tensor_tensor(
                            out=junkw[:], in0=prednat[:, pcols], scalar=0.0,
                            in1=cs[:], op0=mybir.AluOpType.add,
                            op1=mybir.AluOpType.mult,
                            accum_out=dg[:, gi * K17 + slot:gi * K17 + slot + 1],
                        )
                    return emit

                for slot in range(K17):
                    pending.append(mk_dot(slot))

                def mk_stats():
                    dcols = slice(gi * K17, (gi + 1) * K17)
                    nc.vector.tensor_reduce(
                        out=negmax[:, t:t + 1], in_=dg[:, dcols],
                        op=mybir.AluOpType.max, axis=mybir.AxisListType.X,
                        negate=True,
                    )
                    nc.scalar.activation(
                        et17[:, dcols], dg[:, dcols],
                        mybir.ActivationFunctionType.Exp,
                        bias=negmax[:, t:t + 1],
                        scale=1.0,
                        accum_out=sume[:, t:t + 1],
                    )
                    nc.vector.tensor_copy(
                        dots0[:, t:t + 1], dg[:, gi * K17:gi * K17 + 1]
                    )
                pending.append(mk_stats)

            def stage_b(t):
                rows = slice(t * 128, (t + 1) * 128)
                cm = cmts[t]
                masked = mkp.tile([128, R], F32, tag="mk")
                pbs = {}
                for grp in GROUPS:
                    for n in grp:
                        pb = psB.tile([128, ECH], F32, tag="b")
                        pbs[n] = pb
                    for k in range(KD):
                        for n in grp:
                            nc.tensor.matmul(
                                pbs[n][:],
                                lhsT=predT_sb[:, k, rows],
                                rhs=encT_sb[:, k, n * ECH:(n + 1) * ECH],
                                start=(k == 0),
                                stop=(k == KD - 1),
                            )
                    for n in grp:
                        cols = slice(n * ECH, (n + 1) * ECH)
                        nc.vector.tensor_add(masked[:, cols], pbs[n][:], cm[:, cols])
                        nc.vector.tensor_reduce(
                            out=nmx[:, t * 8 + n:t * 8 + n + 1],
                            in_=masked[:, cols],
                            op=mybir.AluOpType.max, axis=mybir.AxisListType.X,
                        )
                        drain(1)
                nc.vector.tensor_reduce(
                    out=negmax[:, t:t + 1], in_=nmx[:, t * 8:t * 8 + NE],
                    op=mybir.AluOpType.max, axis=mybir.AxisListType.X, negate=True,
                )
                et = etp.tile([128, R], BF16, tag="et")
                nc.scalar.activation(
                    et[:], masked[:],
                    mybir.ActivationFunctionType.Exp,
                    bias=negmax[:, t:t + 1],
                    scale=1.0,
                    accum_out=sume[:, t:t + 1],
                )

            stage_a(0)
            diag(0)
            stage_b(0)
            gather_tile(1)
            stage_a(1)
            diag(2)
            stage_b(2)
            gather_tile(3)
            stage_a(2)
            diag(4)
            stage_b(4)
            diag(5)
            stage_b(5)
            stage_a(3)
            diag(6)
            stage_b(6)
            stage_a(4)
            for t in range(7, NT):
                diag(t)
                stage_b(t)
            drain(len(pending))

            # ---- final: loss/corr per prediction, masked, reduced ----
            nc.scalar.activation(lnS[:], sume[:], mybir.ActivationFunctionType.Ln)
            t1 = const.tile([128, NT], F32)
            nc.vector.tensor_sub(t1[:], lnS[:], dots0[:])
            lossp = const.tile([128, NT], F32)
            nc.vector.tensor_sub(lossp[:], t1[:], negmax[:])
            tmax = const.tile([128, NT], F32)
            nc.vector.tensor_scalar_mul(tmax[:], negmax[:], -1.0)
            corrp = const.tile([128, NT], F32)
            nc.vector.tensor_tensor(
                out=corrp[:], in0=dots0[:], in1=tmax[:], op=mybir.AluOpType.is_ge
            )
            nc.vector.tensor_mul(res[:, 0:NT], lossp[:], vmask_sb[:])
            nc.vector.tensor_mul(res[:, NT:2 * NT], corrp[:], vmask_sb[:])

            fin = const.tile([128, 2], F32)
            nc.vector.reduce_sum(fin[:, 0:1], res[:, 0:NT], axis=mybir.AxisListType.X)
            nc.vector.reduce_sum(fin[:, 1:2], res[:, NT:2 * NT], axis=mybir.AxisListType.X)
            pf = psD.tile([1, 2], F32, tag="d")
            nc.tensor.matmul(pf[:], lhsT=ones[:], rhs=fin[:], start=True, stop=True)
            out_sb = const.tile([1, 2], F32)
            nc.vector.tensor_copy(out_sb[:], pf[:])
            nc.sync.dma_start(out=out_d.ap(), in_=out_sb[:])

            if DEBUG:
                nc.sync.dma_start(out=predT_dbg.ap(), in_=predT_sb[:])
                nc.sync.dma_start(out=cols_dbg.ap()[:, 0:NT], in_=dots0[:])
                nc.sync.dma_start(out=cols_dbg.ap()[:, NT:2 * NT], in_=negmax[:])
                nc.sync.dma_start(out=cols_dbg.ap()[:, 2 * NT:3 * NT], in_=sume[:])
                nc.sync.dma_start(out=cols_dbg.ap()[:, 3 * NT:4 * NT], in_=lnS[:])

    nc.compile()
    _CACHE["nc"] = nc
    return nc


def _to_tiled_T(rows_bf16):
    """[N, D] row-major (bf16) -> [128, KD*N] (d_low, (k, p)) layout,
    per-partition contiguous."""
    n = rows_bf16.shape[0]
    return np.ascontiguousarray(
        rows_bf16.T.reshape(KD, 128, n).transpose(1, 0, 2).reshape(128, KD * n)
    )


def _prep_in_maps(contexts, encodings, Wk_w, Wk_b, ctx_idx, cand_idx):
    ctx_flat = np.ascontiguousarray(
        np.asarray(contexts, dtype=np.float32).reshape(R, D)
    ).astype(ml_dtypes.bfloat16)
    enc_flat = np.ascontiguousarray(
        np.asarray(encodings, dtype=np.float32).reshape(R, D)
    ).astype(ml_dtypes.bfloat16)
    encT = np.ascontiguousarray(
        np.asarray(encodings, dtype=np.float32).reshape(R, D).T
    ).astype(ml_dtypes.bfloat16)
    # W^T [din, dout] per step, pre-tiled to [128, KD*D] (per-partition
    # contiguous: partition = din_low, then (din_chunk, dout))
    WTf = np.asarray(Wk_w, dtype=np.float32).transpose(0, 2, 1).astype(ml_dtypes.bfloat16)
    WT = np.ascontiguousarray(
        WTf.reshape(S, KD, 128, D).transpose(0, 2, 1, 3).reshape(S, 128, KD * D)
    )
    biasT = np.ascontiguousarray(
        np.asarray(Wk_b, dtype=np.float32).reshape(S, KD, 128).transpose(2, 0, 1)
        .reshape(128, S * KD)
    )
    ctx_idx = np.asarray(ctx_idx, dtype=np.int32)
    cand_idx = np.asarray(cand_idx, dtype=np.int32)

    offs = np.concatenate([[0], np.cumsum(STEP_LENS)]).astype(np.int64)

    in_maps = []
    for c in range(N_CORES):
        ci_parts, ki_parts = [], []
        for s in range(S):
            a = int(offs[s]) + c * L[s]
            ci_parts.append(ctx_idx[a:a + L[s]])
            ki_parts.append(cand_idx[a:a + L[s]])
        ci = np.concatenate(ci_parts)                          # [1120]
        ki = np.concatenate(ki_parts, axis=0).astype(np.int64)  # [1120, 17]
        ci_pad = np.zeros(PP, np.int64)
        ci_pad[:PC] = ci
        c0_pad = np.zeros(PP, np.int64)
        c0_pad[:PC] = ki[:, 0]
        g = ctx_flat[ci_pad]
        ctxA = _to_tiled_T(g[0:L[0]])
        ctxB = _to_tiled_T(g[L[0]:L[0] + L[1]])
        ctxC = _to_tiled_T(g[L[0] + L[1]:PP])
        c0r = enc_flat[c0_pad]
        c0T = np.ascontiguousarray(
            np.stack([_to_tiled_T(c0r[t * 128:(t + 1) * 128]) for t in range(NT)])
        )
        vmask = np.ascontiguousarray(
            (np.arange(PP) < PC).astype(np.float32).reshape(NT, 128).T
        )
        prow = np.arange(PC)
        mm = np.zeros((PP, R), np.float32)
        np.add.at(mm, (np.repeat(prow, K17), ki.ravel()), 1.0)
        mm[PC:, 0] = 1.0
        with np.errstate(divide="ignore"):
            cm = np.where(mm > 0, np.log(np.maximum(mm, 1.0)), NEGINF).astype(
                np.float32
            )
        cands = {
            t: np.ascontiguousarray(
                enc_flat[ki[t * 128:(t + 1) * 128, :]].transpose(1, 0, 2)
            )
            for t in GT
        }
        in_maps.append(
            {
                **{f"cand{t}": cands[t] for t in GT},
                "ctxA": ctxA,
                "ctxB": ctxB,
                "ctxC": ctxC,
                "c0T": c0T,
                "encT": encT,
                "WT": WT,
                "biasT": biasT,
                "vmask": vmask,
                "cmask": cm.astype(ml_dtypes.bfloat16),
            }
        )
    return in_maps


def _install_ntff_hook():
    """Provide antenv.axon_hooks if the image lacks it, so trace=True can
    capture NTFF profiles through the injected libaxon_pjrt.so."""
    import sys
    import types
    import ctypes
    import contextlib
    import os

    try:
        from antenv.axon_hooks import get_axon_ntff_profile_hook  # noqa: F401

        return
    except ImportError:
        pass
    so_path = "/opt/axon/libaxon_pjrt.so"
    if not os.path.exists(so_path):
        return
    lib = ctypes.CDLL(so_path)
    if not hasattr(lib, "axon_start_nrt_profile"):
        return
    lib.axon_start_nrt_profile.argtypes = [
        ctypes.POINTER(ctypes.c_int64),
        ctypes.c_size_t,
    ]
    lib.axon_start_nrt_profile.restype = ctypes.c_int64
    lib.axon_stop_nrt_profile.argtypes = [ctypes.c_char_p]
    lib.axon_stop_nrt_profile.restype = ctypes.c_int64

    @contextlib.contextmanager
    def _hook(output_dir, device_ids):
        import jax

        jax.devices()
        if device_ids:
            ids = (ctypes.c_int64 * len(device_ids))(*device_ids)
            rc = lib.axon_start_nrt_profile(ids, len(device_ids))
        else:
            rc = lib.axon_start_nrt_profile(None, 0)
        if rc != 0:
            raise RuntimeError(f"axon_start_nrt_profile rc={rc}")
        try:
            yield
        finally:
            n = lib.axon_stop_nrt_profile(str(output_dir).encode())
            print(f"ntff profile: {n} file(s) written to {output_dir}")

    mod = types.ModuleType("antenv.axon_hooks")
    mod.get_axon_ntff_profile_hook = lambda: _hook
    mod.set_axon_ntff_profile_hook = lambda h: None
    sys.modules["antenv.axon_hooks"] = mod


def run(inputs, trace=False, **kwargs):
    """Run the SPMD kernel; returns (loss, correct, BassKernelResults)."""
    if trace:
        _install_ntff_hook()
    nc = _build()
    in_maps = _prep_in_maps(**inputs)
    res = run_bass_kernel_spmd(
        nc, in_maps, core_ids=list(range(N_CORES)), trace=trace, **kwargs
    )
    sums = np.stack([r["out"].reshape(2) for r in res.results])  # [8, 2]
    tot = sums.sum(axis=0, dtype=np.float64)
    loss = np.float32(tot[0] / P_TOTAL)
    correct = np.float32(tot[1] / P_TOTAL)
    return loss, correct, res


def kernel(**inputs):
    loss, correct, _ = run(inputs, trace=False)
    return loss, correct


# revision 15
# speedup vs baseline: 1.0352x; 1.0352x over previous
"""Trainium2 Bass kernel for the CPC loss (nn_CPC_292057776614) — v2.

Data-parallel over the prediction axis (8 cores, 1120 preds each, padded
to 1152 = 9 tiles of 128). The ctx-row gather and the positive-candidate
row gather are done ON HOST (indices are host-known), pre-transposed
into the [d_low, k, p] SBUF layout the PE needs — so the device runs no
indirect DMAs and no xbar transposes (both serialized badly against the
bulk weight/encoding streams in v1).

Per core:
  - stage A: predT[dout, p] = W_s^T-contraction of ctxT, 10 PSUM
    k-accumulated matmuls per (s, m-chunk); bias applied during the ACT
    PSUM->SBUF evacuation (bf16 cast).
  - per tile t (128 preds):
    - dots0 exactly on PE: 10 matmuls predT_t x c0T_t -> PSUM [128,128];
      diagonal extracted with one fused DVE scalar_tensor_tensor
      (identity mask multiply + row-sum accumulator).
    - stage B all-pairs scores vs all 3136 encodings: k-outer matmuls
      (lhsT stationary per k, 3 chunk-groups x {3,2,2} PSUM banks); each
      chunk evacuated by a DVE tensor_add with the additive candidate
      mask C (= ln(multiplicity) on candidate slots / -1e30 elsewhere).
    - negmax via one negated DVE row-max; sumexp via ACT exp with
      accumulate (bias = negmax).
  - loss_p = ln(sumexp) - negmax - dots0 ; corr_p = dots0 >= -negmax;
    masked by the valid mask, reduced via a ones-vector matmul.

Host sums the 8 per-core [loss_sum, correct_sum] pairs / 8960.

DMA schedule: one sync-ring (SP HWDGE) FIFO in need order
  W0, ctxT(3 chunks), encA, cm0, W1, cm1, encB, cm2, W2, cm3, W3, cm4,
  W4, cm5..cm8
plus the small consts and the per-tile c0T prefetches on the scalar
(ACT HWDGE) ring. PE emission interleaves stage-A steps with stage-B
tiles so the weight stream stays ahead of compute.
"""

import numpy as np
import ml_dtypes

import concourse.bass as bass
import concourse.mybir as mybir
import concourse.tile as tile
from concourse import bacc
from concourse.bass_utils import run_bass_kernel_spmd
from concourse.masks import make_identity

BF16 = mybir.dt.bfloat16
F32 = mybir.dt.float32
I32 = mybir.dt.int32

# Problem constants (hardcoded; kernel.py must be self-contained).
B, G, D, S, NEG = 64, 7, 1280, 5, 16
CELLS = G * G            # 49
R = B * CELLS            # 3136 rows in ctx/enc
K17 = NEG + 1            # 17 candidates per prediction
STEP_LENS = [B * (G - 1 - s) * G for s in range(S)]     # [2688,2240,1792,1344,896]
P_TOTAL = sum(STEP_LENS)                                # 8960
N_CORES = 8
L = [sl // N_CORES for sl in STEP_LENS]                 # [336,280,224,168,112]
PC = sum(L)                                             # 1120 per core
NT = 9                                                  # p-tiles of 128
PP = NT * 128                                           # 1152 padded
PO = [sum(L[:s]) for s in range(S)]                     # per-core step offsets
KD = D // 128                                           # 10 k-tiles
ECH = 448                                               # e-chunk width (448*7=3136)
NE = R // ECH                                           # 7 chunks
GROUPS = [(0, 1, 2), (3, 4), (5, 6)]                    # stage-B chunk groups
CW2 = PP - L[0] - L[1]                                  # 536: s2+s3+s4+pad ctx cols
NEGINF = -1.0e30
NWARM = 200
GT = (1, 3)                                             # gather-path tiles
APT = tuple(t for t in range(NT) if t not in GT)        # all-pairs tiles                                             # HAM warmup matmuls

_CACHE = {}

DEBUG = bool(int(__import__("os").environ.get("BASS_CPC_DEBUG", "0")))


def _build():
    if "nc" in _CACHE:
        return _CACHE["nc"]

    nc = bacc.Bacc("TRN2", target_bir_lowering=False, debug=False)

    ctxA_d = nc.dram_tensor("ctxA", [128, KD * L[0]], BF16, kind="ExternalInput")
    ctxB_d = nc.dram_tensor("ctxB", [128, KD * L[1]], BF16, kind="ExternalInput")
    ctxC_d = nc.dram_tensor("ctxC", [128, KD * CW2], BF16, kind="ExternalInput")
    c0T_d = nc.dram_tensor("c0T", [NT, 128, KD * 128], BF16, kind="ExternalInput")
    encT_d = nc.dram_tensor("encT", [D, R], BF16, kind="ExternalInput")
    WT_d = nc.dram_tensor("WT", [S, 128, KD * D], BF16, kind="ExternalInput")
    bias_d = nc.dram_tensor("biasT", [128, S * KD], F32, kind="ExternalInput")
    vmask_d = nc.dram_tensor("vmask", [128, NT], F32, kind="ExternalInput")
    cmask_d = nc.dram_tensor("cmask", [PP, R], BF16, kind="ExternalInput")
    cand_d = {
        t: nc.dram_tensor(f"cand{t}", [K17, 128, D], BF16, kind="ExternalInput")
        for t in GT
    }
    out_d = nc.dram_tensor("out", [1, 2], F32, kind="ExternalOutput")
    if DEBUG:
        predT_dbg = nc.dram_tensor("predT_dbg", [128, KD, PP], BF16, kind="ExternalOutput")
        cols_dbg = nc.dram_tensor("cols_dbg", [128, 4 * NT], F32, kind="ExternalOutput")

    with tile.TileContext(nc) as tc:
        with (
            tc.tile_pool(name="const", bufs=1) as const,
            tc.tile_pool(name="c0tp", bufs=2) as c0tp,
            tc.tile_pool(name="wpool", bufs=2) as wpool,
            tc.tile_pool(name="cmp", bufs=2) as cmp,
            tc.tile_pool(name="mkp", bufs=1) as mkp,
            tc.tile_pool(name="candp", bufs=2) as candp,
            tc.tile_pool(name="etp", bufs=1) as etp,
            tc.tile_pool(name="psA", bufs=2, space="PSUM") as psA,
            tc.tile_pool(name="psB", bufs=5, space="PSUM") as psB,
            tc.tile_pool(name="psD", bufs=1, space="PSUM") as psD,
        ):
            # ---- small consts (scalar HWDGE ring, first) ----
            bias_sb = const.tile([128, S * KD], F32)
            nc.scalar.dma_start(out=bias_sb[:], in_=bias_d.ap())
            vmask_sb = const.tile([128, NT], F32)
            nc.scalar.dma_start(out=vmask_sb[:], in_=vmask_d.ap())

            identB = const.tile([128, 128], BF16)
            make_identity(nc, identB[:])
            ones = const.tile([128, 1], F32)
            nc.vector.memset(ones[:], 1.0)

            # HAM warmup: keep the PE busy during the initial load window so
            # the clock gate opens before real compute starts. Accumulating
            # chain (DCE-safe); result kept alive by a copy nobody uses.
            wps = psD.tile([128, 128], F32, tag="d")
            for i in range(NWARM):
                nc.tensor.matmul(
                    wps[:], lhsT=identB[:], rhs=identB[:],
                    start=(i == 0), stop=(i == NWARM - 1),
                )
            warmkeep = const.tile([128, 1], F32)
            nc.vector.tensor_copy(warmkeep[:], wps[:, 0:1])

            encT_sb = const.tile([128, KD, R], BF16)
            ctxA_sb = const.tile([128, KD, L[0]], BF16)
            ctxB_sb = const.tile([128, KD, L[1]], BF16)
            ctxC_sb = const.tile([128, KD, CW2], BF16)
            predT_sb = const.tile([128, KD, PP], BF16)
            nc.vector.memset(predT_sb[:, :, PC:PP], 0.0)

            negmax = const.tile([128, NT], F32)
            nmx = const.tile([128, NT * 8], F32)
            dots0 = const.tile([128, NT], F32)
            sume = const.tile([128, NT], F32)
            lnS = const.tile([128, NT], F32)
            junkw = const.tile([128, D], BF16)
            res = const.tile([128, 2 * NT], F32)
            dg = const.tile([128, len(GT) * K17], F32)
            et17 = const.tile([128, len(GT) * K17], BF16)
            prednat = const.tile([128, len(GT) * D], BF16)

            # ---- c0T per-tile prefetch (scalar ring) ----
            c0ts = {}

            def emit_c0t(t):
                ct = c0tp.tile([128, KD, 128], BF16, tag="c0t")
                nc.scalar.dma_start(
                    out=ct[:],
                    in_=c0T_d.ap()[t].rearrange("p (k j) -> p k j", k=KD),
                )
                c0ts[t] = ct

            emit_c0t(APT[0])
            emit_c0t(APT[1])

            # ---- big streams (sync HWDGE ring) in need order ----
            wtiles = {}

            def emit_w(s):
                w = wpool.tile([128, KD, D], BF16, tag="w")
                wsrc = WT_d.ap()[s].rearrange("p (k j) -> p k j", k=KD)
                nc.sync.dma_start(out=w[:], in_=wsrc)
                wtiles[s] = w

            cmts = {}

            def emit_cm(t):
                cm = cmp.tile([128, R], BF16, tag="cm")
                nc.sync.dma_start(
                    out=cm[:], in_=cmask_d.ap()[t * 128:(t + 1) * 128, :]
                )
                cmts[t] = cm

            encsrc = encT_d.ap().rearrange("(k p) e -> p k e", p=128)
            nc.sync.dma_start(
                out=ctxA_sb[:], in_=ctxA_d.ap().rearrange("p (k j) -> p k j", k=KD)
            )
            emit_w(0)
            nc.sync.dma_start(
                out=ctxB_sb[:], in_=ctxB_d.ap().rearrange("p (k j) -> p k j", k=KD)
            )
            nc.sync.dma_start(
                out=ctxC_sb[:], in_=ctxC_d.ap().rearrange("p (k j) -> p k j", k=KD)
            )
            nc.sync.dma_start(out=encT_sb[:, :, 0:4 * ECH], in_=encsrc[:, :, 0:4 * ECH])
            emit_cm(0)
            emit_w(1)
            nc.sync.dma_start(out=encT_sb[:, :, 4 * ECH:R], in_=encsrc[:, :, 4 * ECH:R])
            emit_cm(2)
            emit_w(2)
            emit_w(3)
            emit_cm(4)
            emit_w(4)
            for t in range(5, NT):
                emit_cm(t)

            # ---- compute emission: interleave A-steps, diag, B-tiles ----
            CTX_OF = {0: None, 1: None, 2: None, 3: None, 4: None}

            def _ctx_rhs(s, k):
                if s == 0:
                    return ctxA_sb[:, k, :]
                if s == 1:
                    return ctxB_sb[:, k, :]
                off = PO[s] - PO[2]
                return ctxC_sb[:, k, off:off + L[s]]

            def stage_a(s):
                w = wtiles[s]
                lo, ln = PO[s], L[s]
                for m in range(KD):
                    pa = psA.tile([128, ECH], F32, tag="a")
                    for k in range(KD):
                        nc.tensor.matmul(
                            pa[:, :ln],
                            lhsT=w[:, k, m * 128:(m + 1) * 128],
                            rhs=_ctx_rhs(s, k),
                            start=(k == 0),
                            stop=(k == KD - 1),
                        )
                    nc.scalar.activation(
                        predT_sb[:, m, lo:lo + ln],
                        pa[:, :ln],
                        mybir.ActivationFunctionType.Identity,
                        bias=bias_sb[:, s * KD + m:s * KD + m + 1],
                        scale=1.0,
                    )
                    drain(1)

            def diag(t):
                ct = c0ts[t]
                i = APT.index(t)
                if i + 2 < len(APT):
                    emit_c0t(APT[i + 2])
                rows = slice(t * 128, (t + 1) * 128)
                pd = psD.tile([128, 128], F32, tag="d")
                for k in range(KD):
                    nc.tensor.matmul(
                        pd[:],
                        lhsT=predT_sb[:, k, rows],
                        rhs=ct[:, k, :],
                        start=(k == 0),
                        stop=(k == KD - 1),
                    )
                nc.vector.scalar_tensor_tensor(
                    out=junkw[:, 0:128], in0=pd[:], scalar=0.0, in1=identB[:],
                    op0=mybir.AluOpType.add, op1=mybir.AluOpType.mult,
                    accum_out=dots0[:, t:t + 1],
                )

            pending = []

            def drain(n):
                for _ in range(min(n, len(pending))):
                    pending.pop(0)()

            def gather_tile(t):
                gi = GT.index(t)
                rows = slice(t * 128, (t + 1) * 128)
                pcols = slice(gi * D, (gi + 1) * D)
                # transpose predT tile -> natural [p, d] via PE (psA: 2 banks)
                for k in range(KD):
                    pt = psA.tile([128, 128], BF16, tag="a")
                    nc.tensor.transpose(
                        pt[:], predT_sb[:, k, rows], identB[:]
                    )
                    nc.vector.tensor_copy(
                        prednat[:, gi * D + k * 128:gi * D + (k + 1) * 128], pt[:]
                    )
                # queue the 17 fused dot ops + stats (drained between evacs);
                # cand DMAs dispatched one slot ahead of their dot op
                state = {"dma": 0, "tiles": {}}

                def _dispatch():
                    s = state["dma"]
                    if s < K17:
                        cs = candp.tile([128, D], BF16, tag="cand")
                        nc.gpsimd.dma_start(out=cs[:], in_=cand_d[t].ap()[s])
                        state["tiles"][s] = cs
                        state["dma"] = s + 1

                def mk_dot(slot):
                    def emit():
                        if slot == 0:
                            _dispatch()
                        _dispatch()
                        cs = state["tiles"].pop(slot)
                        nc.vector.scalar_tensor_tensor(
                            out=junkw[:], in0=prednat[:, pcols], scalar=0.0,
                            in1=cs[:], op0=mybir.AluOpType.add,
                            op1=mybir.AluOpType.mult,
                            accum_out=dg[:, gi * K17 + slot:gi * K17 + slot + 1],
                        )
                    return emit

                for slot in range(K17):
                    pending.append(mk_dot(slot))

                def mk_stats():
                    dcols = slice(gi * K17, (gi + 1) * K17)
                    nc.vector.tensor_reduce(
                        out=negmax[:, t:t + 1], in_=dg[:, dcols],
                        op=mybir.AluOpType.max, axis=mybir.AxisListType.X,
                        negate=True,
                    )
                    nc.scalar.activation(
                        et17[:, dcols], dg[:, dcols],
                        mybir.ActivationFunctionType.Exp,
                        bias=negmax[:, t:t + 1],
                        scale=1.0,
                        accum_out=sume[:, t:t + 1],
                    )
                    nc.vector.tensor_copy(
                        dots0[:, t:t + 1], dg[:, gi * K17:gi * K17 + 1]
                    )
                pending.append(mk_stats)

            def stage_b(t):
                rows = slice(t * 128, (t + 1) * 128)
                cm = cmts[t]
                masked = mkp.tile([128, R], F32, tag="mk")
                pbs = {}
                for grp in GROUPS:
                    for n in grp:
                        pb = psB.tile([128, ECH], F32, tag="b")
                        pbs[n] = pb
                    for k in range(KD):
                        for n in grp:
                            nc.tensor.matmul(
                                pbs[n][:],
                                lhsT=predT_sb[:, k, rows],
                                rhs=encT_sb[:, k, n * ECH:(n + 1) * ECH],
                                start=(k == 0),
                                stop=(k == KD - 1),
                            )
                    for n in grp:
                        cols = slice(n * ECH, (n + 1) * ECH)
                        nc.vector.tensor_add(masked[:, cols], pbs[n][:], cm[:, cols])
                        nc.vector.tensor_reduce(
                            out=nmx[:, t * 8 + n:t * 8 + n + 1],
                            in_=masked[:, cols],
                            op=mybir.AluOpType.max, axis=mybir.AxisListType.X,
                        )
                        drain(1)
                nc.vector.tensor_reduce(
                    out=negmax[:, t:t + 1], in_=nmx[:, t * 8:t * 8 + NE],
                    op=mybir.AluOpType.max, axis=mybir.AxisListType.X, negate=True,
                )
                et = etp.tile([128, R], BF16, tag="et")
                nc.scalar.activation(
                    et[:], masked[:],
                    mybir.ActivationFunctionType.Exp,
                    bias=negmax[:, t:t + 1],
                    scale=1.0,
                    accum_out=sume[:, t:t + 1],
                )

            stage_a(0)
            diag(0)
            stage_b(0)
            gather_tile(1)
            stage_a(1)
            diag(2)
            stage_b(2)
            gather_tile(3)
            stage_a(2)
            diag(4)
            stage_b(4)
            diag(5)
            stage_b(5)
            stage_a(3)
            diag(6)
            stage_b(6)
            stage_a(4)
            for t in range(7, NT):
                diag(t)
                stage_b(t)
            drain(len(pending))

            # ---- final: loss/corr per prediction, masked, reduced ----
            nc.scalar.activation(lnS[:], sume[:], mybir.ActivationFunctionType.Ln)
            t1 = const.tile([128, NT], F32)
            nc.vector.tensor_sub(t1[:], lnS[:], dots0[:])
            lossp = const.tile([128, NT], F32)
            nc.vector.tensor_sub(lossp[:], t1[:], negmax[:])
            tmax = const.tile([128, NT], F32)
            nc.vector.tensor_scalar_mul(tmax[:], negmax[:], -1.0)
            corrp = const.tile([128, NT], F32)
            nc.vector.tensor_tensor(
                out=corrp[:], in0=dots0[:], in1=tmax[:], op=mybir.AluOpType.is_ge
            )
            nc.vector.tensor_mul(res[:, 0:NT], lossp[:], vmask_sb[:])
            nc.vector.tensor_mul(res[:, NT:2 * NT], corrp[:], vmask_sb[:])

            fin = const.tile([128, 2], F32)
            nc.vector.reduce_sum(fin[:, 0:1], res[:, 0:NT], axis=mybir.AxisListType.X)
            nc.vector.reduce_sum(fin[:, 1:2], res[:, NT:2 * NT], axis=mybir.AxisListType.X)
            pf = psD.tile([1, 2], F32, tag="d")
            nc.tensor.matmul(pf[:], lhsT=ones[:], rhs=fin[:], start=True, stop=True)
            out_sb = const.tile([1, 2], F32)
            nc.vector.tensor_copy(out_sb[:], pf[:])
            nc.sync.dma_start(out=out_d.ap(), in_=out_sb[:])

            if DEBUG:
                nc.sync.dma_start(out=predT_dbg.ap(), in_=predT_sb[:])
                nc.sync.dma_start(out=cols_dbg.ap()[:, 0:NT], in_=dots0[:])
                nc.sync.dma_start(out=cols_dbg.ap()[:, NT:2 * NT], in_=negmax[:])
                nc.sync.dma_start(out=cols_dbg.ap()[:, 2 * NT:3 * NT], in_=sume[:])
                nc.sync.dma_start(out=cols_dbg.ap()[:, 3 * NT:4 * NT], in_=lnS[:])

    nc.compile()
    _CACHE["nc"] = nc
    return nc


def _to_tiled_T(rows_bf16):
    """[N, D] row-major (bf16) -> [128, KD*N] (d_low, (k, p)) layout,
    per-partition contiguous."""
    n = rows_bf16.shape[0]
    return np.ascontiguousarray(
        rows_bf16.T.reshape(KD, 128, n).transpose(1, 0, 2).reshape(128, KD * n)
    )


def _prep_in_maps(contexts, encodings, Wk_w, Wk_b, ctx_idx, cand_idx):
    ctx_flat = np.ascontiguousarray(
        np.asarray(contexts, dtype=np.float32).reshape(R, D)
    ).astype(ml_dtypes.bfloat16)
    enc_flat = np.ascontiguousarray(
        np.asarray(encodings, dtype=np.float32).reshape(R, D)
    ).astype(ml_dtypes.bfloat16)
    encT = np.ascontiguousarray(
        np.asarray(encodings, dtype=np.float32).reshape(R, D).T
    ).astype(ml_dtypes.bfloat16)
    # W^T [din, dout] per step, pre-tiled to [128, KD*D] (per-partition
    # contiguous: partition = din_low, then (din_chunk, dout))
    WTf = np.asarray(Wk_w, dtype=np.float32).transpose(0, 2, 1).astype(ml_dtypes.bfloat16)
    WT = np.ascontiguousarray(
        WTf.reshape(S, KD, 128, D).transpose(0, 2, 1, 3).reshape(S, 128, KD * D)
    )
    biasT = np.ascontiguousarray(
        np.asarray(Wk_b, dtype=np.float32).reshape(S, KD, 128).transpose(2, 0, 1)
        .reshape(128, S * KD)
    )
    ctx_idx = np.asarray(ctx_idx, dtype=np.int32)
    cand_idx = np.asarray(cand_idx, dtype=np.int32)

    offs = np.concatenate([[0], np.cumsum(STEP_LENS)]).astype(np.int64)

    in_maps = []
    for c in range(N_CORES):
        ci_parts, ki_parts = [], []
        for s in range(S):
            a = int(offs[s]) + c * L[s]
            ci_parts.append(ctx_idx[a:a + L[s]])
            ki_parts.append(cand_idx[a:a + L[s]])
        ci = np.concatenate(ci_parts)                          # [1120]
        ki = np.concatenate(ki_parts, axis=0).astype(np.int64)  # [1120, 17]
        ci_pad = np.zeros(PP, np.int64)
        ci_pad[:PC] = ci
        c0_pad = np.zeros(PP, np.int64)
        c0_pad[:PC] = ki[:, 0]
        g = ctx_flat[ci_pad]
        ctxA = _to_tiled_T(g[0:L[0]])
        ctxB = _to_tiled_T(g[L[0]:L[0] + L[1]])
        ctxC = _to_tiled_T(g[L[0] + L[1]:PP])
        c0r = enc_flat[c0_pad]
        c0T = np.ascontiguousarray(
            np.stack([_to_tiled_T(c0r[t * 128:(t + 1) * 128]) for t in range(NT)])
        )
        vmask = np.ascontiguousarray(
            (np.arange(PP) < PC).astype(np.float32).reshape(NT, 128).T
        )
        prow = np.arange(PC)
        mm = np.zeros((PP, R), np.float32)
        np.add.at(mm, (np.repeat(prow, K17), ki.ravel()), 1.0)
        mm[PC:, 0] = 1.0
        with np.errstate(divide="ignore"):
            cm = np.where(mm > 0, np.log(np.maximum(mm, 1.0)), NEGINF).astype(
                np.float32
            )
        cands = {
            t: np.ascontiguousarray(
                enc_flat[ki[t * 128:(t + 1) * 128, :]].transpose(1, 0, 2)
            )
            for t in GT
        }
        in_maps.append(
            {
                **{f"cand{t}": cands[t] for t in GT},
                "ctxA": ctxA,
                "ctxB": ctxB,
                "ctxC": ctxC,
                "c0T": c0T,
                "encT": encT,
                "WT": WT,
                "biasT": biasT,
                "vmask": vmask,
                "cmask": cm.astype(ml_dtypes.bfloat16),
            }
        )
    return in_maps


def _install_ntff_hook():
    """Provide antenv.axon_hooks if the image lacks it, so trace=True can
    capture NTFF profiles through the injected libaxon_pjrt.so."""
    import sys
    import types
    import ctypes
    import contextlib
    import os

    try:
        from antenv.axon_hooks import get_axon_ntff_profile_hook  # noqa: F401

        return
    except ImportError:
        pass
    so_path = "/opt/axon/libaxon_pjrt.so"
    if not os.path.exists(so_path):
        return
    lib = ctypes.CDLL(so_path)
    if not hasattr(lib, "axon_start_nrt_profile"):
        return
    lib.axon_start_nrt_profile.argtypes = [
        ctypes.POINTER(ctypes.c_int64),
        ctypes.c_size_t,
    ]
    lib.axon_start_nrt_profile.restype = ctypes.c_int64
    lib.axon_stop_nrt_profile.argtypes = [ctypes.c_char_p]
    lib.axon_stop_nrt_profile.restype = ctypes.c_int64

    @contextlib.contextmanager
    def _hook(output_dir, device_ids):
        import jax

        jax.devices()
        if device_ids:
            ids = (ctypes.c_int64 * len(device_ids))(*device_ids)
            rc = lib.axon_start_nrt_profile(ids, len(device_ids))
        else:
            rc = lib.axon_start_nrt_profile(None, 0)
        if rc != 0:
            raise RuntimeError(f"axon_start_nrt_profile rc={rc}")
        try:
            yield
        finally:
            n = lib.axon_stop_nrt_profile(str(output_dir).encode())
            print(f"ntff profile: {n} file(s) written to {output_dir}")

    mod = types.ModuleType("antenv.axon_hooks")
    mod.get_axon_ntff_profile_hook = lambda: _hook
    mod.set_axon_ntff_profile_hook = lambda h: None
    sys.modules["antenv.axon_hooks"] = mod


def run(inputs, trace=False, **kwargs):
    """Run the SPMD kernel; returns (loss, correct, BassKernelResults)."""
    if trace:
        _install_ntff_hook()
    nc = _build()
    in_maps = _prep_in_maps(**inputs)
    res = run_bass_kernel_spmd(
        nc, in_maps, core_ids=list(range(N_CORES)), trace=trace, **kwargs
    )
    sums = np.stack([r["out"].reshape(2) for r in res.results])  # [8, 2]
    tot = sums.sum(axis=0, dtype=np.float64)
    loss = np.float32(tot[0] / P_TOTAL)
    correct = np.float32(tot[1] / P_TOTAL)
    return loss, correct, res


def kernel(**inputs):
    loss, correct, _ = run(inputs, trace=False)
    return loss, correct


# revision 17
# speedup vs baseline: 1.0515x; 1.0157x over previous
"""Trainium2 Bass kernel for the CPC loss (nn_CPC_292057776614) — v2.

Data-parallel over the prediction axis (8 cores, 1120 preds each, padded
to 1152 = 9 tiles of 128). The ctx-row gather and the positive-candidate
row gather are done ON HOST (indices are host-known), pre-transposed
into the [d_low, k, p] SBUF layout the PE needs — so the device runs no
indirect DMAs and no xbar transposes (both serialized badly against the
bulk weight/encoding streams in v1).

Per core:
  - stage A: predT[dout, p] = W_s^T-contraction of ctxT, 10 PSUM
    k-accumulated matmuls per (s, m-chunk); bias applied during the ACT
    PSUM->SBUF evacuation (bf16 cast).
  - per tile t (128 preds):
    - dots0 exactly on PE: 10 matmuls predT_t x c0T_t -> PSUM [128,128];
      diagonal extracted with one fused DVE scalar_tensor_tensor
      (identity mask multiply + row-sum accumulator).
    - stage B all-pairs scores vs all 3136 encodings: k-outer matmuls
      (lhsT stationary per k, 3 chunk-groups x {3,2,2} PSUM banks); each
      chunk evacuated by a DVE tensor_add with the additive candidate
      mask C (= ln(multiplicity) on candidate slots / -1e30 elsewhere).
    - negmax via one negated DVE row-max; sumexp via ACT exp with
      accumulate (bias = negmax).
  - loss_p = ln(sumexp) - negmax - dots0 ; corr_p = dots0 >= -negmax;
    masked by the valid mask, reduced via a ones-vector matmul.

Host sums the 8 per-core [loss_sum, correct_sum] pairs / 8960.

DMA schedule: one sync-ring (SP HWDGE) FIFO in need order
  W0, ctxT(3 chunks), encA, cm0, W1, cm1, encB, cm2, W2, cm3, W3, cm4,
  W4, cm5..cm8
plus the small consts and the per-tile c0T prefetches on the scalar
(ACT HWDGE) ring. PE emission interleaves stage-A steps with stage-B
tiles so the weight stream stays ahead of compute.
"""

import numpy as np
import ml_dtypes

import concourse.bass as bass
import concourse.mybir as mybir
import concourse.tile as tile
from concourse import bacc
from concourse.bass_utils import run_bass_kernel_spmd
from concourse.masks import make_identity

BF16 = mybir.dt.bfloat16
F32 = mybir.dt.float32
I32 = mybir.dt.int32

# Problem constants (hardcoded; kernel.py must be self-contained).
B, G, D, S, NEG = 64, 7, 1280, 5, 16
CELLS = G * G            # 49
R = B * CELLS            # 3136 rows in ctx/enc
K17 = NEG + 1            # 17 candidates per prediction
STEP_LENS = [B * (G - 1 - s) * G for s in range(S)]     # [2688,2240,1792,1344,896]
P_TOTAL = sum(STEP_LENS)                                # 8960
N_CORES = 8
L = [sl // N_CORES for sl in STEP_LENS]                 # [336,280,224,168,112]
PC = sum(L)                                             # 1120 per core
NT = 9                                                  # p-tiles of 128
PP = NT * 128                                           # 1152 padded
PO = [sum(L[:s]) for s in range(S)]                     # per-core step offsets
KD = D // 128                                           # 10 k-tiles
ECH = 448                                               # e-chunk width (448*7=3136)
NE = R // ECH                                           # 7 chunks
GROUPS = [(0, 1, 2), (3, 4), (5, 6)]                    # stage-B chunk groups
CW2 = PP - L[0] - L[1]                                  # 536: s2+s3+s4+pad ctx cols
NEGINF = -1.0e30
NWARM = 200                                             # HAM warmup matmuls

_CACHE = {}

DEBUG = bool(int(__import__("os").environ.get("BASS_CPC_DEBUG", "0")))


def _build():
    if "nc" in _CACHE:
        return _CACHE["nc"]

    nc = bacc.Bacc("TRN2", target_bir_lowering=False, debug=False)

    ctxA_d = nc.dram_tensor("ctxA", [128, KD * L[0]], BF16, kind="ExternalInput")
    ctxB_d = nc.dram_tensor("ctxB", [128, KD * L[1]], BF16, kind="ExternalInput")
    ctxC_d = nc.dram_tensor("ctxC", [128, KD * CW2], BF16, kind="ExternalInput")
    c0T_d = nc.dram_tensor("c0T", [NT, 128, KD * 128], BF16, kind="ExternalInput")
    encT_d = nc.dram_tensor("encT", [D, R], BF16, kind="ExternalInput")
    WT_d = nc.dram_tensor("WT", [S, 128, KD * D], BF16, kind="ExternalInput")
    bias_d = nc.dram_tensor("biasT", [128, S * KD], F32, kind="ExternalInput")
    vmask_d = nc.dram_tensor("vmask", [128, NT], F32, kind="ExternalInput")
    cmask_d = nc.dram_tensor("cmask", [PP, R], BF16, kind="ExternalInput")
    out_d = nc.dram_tensor("out", [1, 2], F32, kind="ExternalOutput")
    if DEBUG:
        predT_dbg = nc.dram_tensor("predT_dbg", [128, KD, PP], BF16, kind="ExternalOutput")
        cols_dbg = nc.dram_tensor("cols_dbg", [128, 4 * NT], F32, kind="ExternalOutput")

    with tile.TileContext(nc) as tc:
        with (
            tc.tile_pool(name="const", bufs=1) as const,
            tc.tile_pool(name="c0tp", bufs=2) as c0tp,
            tc.tile_pool(name="wpool", bufs=2) as wpool,
            tc.tile_pool(name="cmp", bufs=2) as cmp,
            tc.tile_pool(name="mkp", bufs=1) as mkp,
            tc.tile_pool(name="etp", bufs=1) as etp,
            tc.tile_pool(name="psA", bufs=2, space="PSUM") as psA,
            tc.tile_pool(name="psB", bufs=5, space="PSUM") as psB,
            tc.tile_pool(name="psD", bufs=1, space="PSUM") as psD,
        ):
            # ---- small consts (scalar HWDGE ring, first) ----
            bias_sb = const.tile([128, S * KD], F32)
            nc.scalar.dma_start(out=bias_sb[:], in_=bias_d.ap())
            vmask_sb = const.tile([128, NT], F32)
            nc.scalar.dma_start(out=vmask_sb[:], in_=vmask_d.ap())

            identB = const.tile([128, 128], BF16)
            make_identity(nc, identB[:])
            ones = const.tile([128, 1], F32)
            nc.vector.memset(ones[:], 1.0)

            # HAM warmup: keep the PE busy during the initial load window so
            # the clock gate opens before real compute starts. Accumulating
            # chain (DCE-safe); result kept alive by a copy nobody uses.
            wps = psD.tile([128, 128], F32, tag="d")
            for i in range(NWARM):
                nc.tensor.matmul(
                    wps[:], lhsT=identB[:], rhs=identB[:],
                    start=(i == 0), stop=(i == NWARM - 1),
                )
            warmkeep = const.tile([128, 1], F32)
            nc.vector.tensor_copy(warmkeep[:], wps[:, 0:1])

            encT_sb = const.tile([128, KD, R], BF16)
            ctxA_sb = const.tile([128, KD, L[0]], BF16)
            ctxB_sb = const.tile([128, KD, L[1]], BF16)
            ctxC_sb = const.tile([128, KD, CW2], BF16)
            predT_sb = const.tile([128, KD, PP], BF16)
            nc.vector.memset(predT_sb[:, :, PC:PP], 0.0)

            negmax = const.tile([128, NT], F32)
            nmx = const.tile([128, NT * 8], F32)
            dots0 = const.tile([128, NT], F32)
            sume = const.tile([128, NT], F32)
            lnS = const.tile([128, NT], F32)
            junk128 = const.tile([128, 128], F32)
            res = const.tile([128, 2 * NT], F32)

            # ---- c0T per-tile prefetch (scalar ring) ----
            c0ts = {}

            def emit_c0t(t):
                ct = c0tp.tile([128, KD, 128], BF16, tag="c0t")
                nc.scalar.dma_start(
                    out=ct[:],
                    in_=c0T_d.ap()[t].rearrange("p (k j) -> p k j", k=KD),
                )
                c0ts[t] = ct

            emit_c0t(0)
            emit_c0t(1)

            # ---- big streams (sync HWDGE ring) in need order ----
            wtiles = {}

            def emit_w(s):
                w = wpool.tile([128, KD, D], BF16, tag="w")
                wsrc = WT_d.ap()[s].rearrange("p (k j) -> p k j", k=KD)
                nc.sync.dma_start(out=w[:], in_=wsrc)
                wtiles[s] = w

            cmts = {}

            def emit_cm(t):
                cm = cmp.tile([128, R], BF16, tag="cm")
                nc.sync.dma_start(
                    out=cm[:], in_=cmask_d.ap()[t * 128:(t + 1) * 128, :]
                )
                cmts[t] = cm

            encsrc = encT_d.ap().rearrange("(k p) e -> p k e", p=128)
            nc.sync.dma_start(
                out=ctxA_sb[:], in_=ctxA_d.ap().rearrange("p (k j) -> p k j", k=KD)
            )
            emit_w(0)
            nc.sync.dma_start(
                out=ctxB_sb[:], in_=ctxB_d.ap().rearrange("p (k j) -> p k j", k=KD)
            )
            nc.sync.dma_start(out=encT_sb[:, :, 0:3 * ECH], in_=encsrc[:, :, 0:3 * ECH])
            emit_cm(0)
            emit_w(1)
            nc.sync.dma_start(
                out=ctxC_sb[:], in_=ctxC_d.ap().rearrange("p (k j) -> p k j", k=KD)
            )
            nc.sync.dma_start(out=encT_sb[:, :, 3 * ECH:5 * ECH], in_=encsrc[:, :, 3 * ECH:5 * ECH])
            nc.sync.dma_start(out=encT_sb[:, :, 5 * ECH:R], in_=encsrc[:, :, 5 * ECH:R])
            emit_cm(1)
            emit_w(2)
            emit_cm(2)
            emit_w(3)
            emit_cm(3)
            emit_w(4)
            for t in range(4, NT):
                emit_cm(t)

            # ---- compute emission: interleave A-steps, diag, B-tiles ----
            CTX_OF = {0: None, 1: None, 2: None, 3: None, 4: None}

            def _ctx_rhs(s, k):
                if s == 0:
                    return ctxA_sb[:, k, :]
                if s == 1:
                    return ctxB_sb[:, k, :]
                off = PO[s] - PO[2]
                return ctxC_sb[:, k, off:off + L[s]]

            def stage_a(s):
                w = wtiles[s]
                lo, ln = PO[s], L[s]
                for m in range(KD):
                    pa = psA.tile([128, ECH], F32, tag="a")
                    for k in range(KD):
                        nc.tensor.matmul(
                            pa[:, :ln],
                            lhsT=w[:, k, m * 128:(m + 1) * 128],
                            rhs=_ctx_rhs(s, k),
                            start=(k == 0),
                            stop=(k == KD - 1),
                        )
                    nc.scalar.activation(
                        predT_sb[:, m, lo:lo + ln],
                        pa[:, :ln],
                        mybir.ActivationFunctionType.Identity,
                        bias=bias_sb[:, s * KD + m:s * KD + m + 1],
                        scale=1.0,
                    )

            def diag(t):
                ct = c0ts[t]
                if t + 2 < NT:
                    emit_c0t(t + 2)
                rows = slice(t * 128, (t + 1) * 128)
                pd = psD.tile([128, 128], F32, tag="d")
                for k in range(KD):
                    nc.tensor.matmul(
                        pd[:],
                        lhsT=predT_sb[:, k, rows],
                        rhs=ct[:, k, :],
                        start=(k == 0),
                        stop=(k == KD - 1),
                    )
                nc.vector.scalar_tensor_tensor(
                    out=junk128[:], in0=pd[:], scalar=0.0, in1=identB[:],
                    op0=mybir.AluOpType.add, op1=mybir.AluOpType.mult,
                    accum_out=dots0[:, t:t + 1],
                )

            def stage_b(t):
                rows = slice(t * 128, (t + 1) * 128)
                cm = cmts[t]
                masked = mkp.tile([128, R], F32, tag="mk")
                pbs = {}
                for grp in GROUPS:
                    for n in grp:
                        pb = psB.tile([128, ECH], F32, tag="b")
                        pbs[n] = pb
                    for k in range(KD):
                        for n in grp:
                            nc.tensor.matmul(
                                pbs[n][:],
                                lhsT=predT_sb[:, k, rows],
                                rhs=encT_sb[:, k, n * ECH:(n + 1) * ECH],
                                start=(k == 0),
                                stop=(k == KD - 1),
                            )
                    for n in grp:
                        cols = slice(n * ECH, (n + 1) * ECH)
                        nc.vector.tensor_add(masked[:, cols], pbs[n][:], cm[:, cols])
                        nc.vector.tensor_reduce(
                            out=nmx[:, t * 8 + n:t * 8 + n + 1],
                            in_=masked[:, cols],
                            op=mybir.AluOpType.max, axis=mybir.AxisListType.X,
                        )
                nc.vector.tensor_reduce(
                    out=negmax[:, t:t + 1], in_=nmx[:, t * 8:t * 8 + NE],
                    op=mybir.AluOpType.max, axis=mybir.AxisListType.X, negate=True,
                )
                et = etp.tile([128, R], BF16, tag="et")
                nc.scalar.activation(
                    et[:], masked[:],
                    mybir.ActivationFunctionType.Exp,
                    bias=negmax[:, t:t + 1],
                    scale=1.0,
                    accum_out=sume[:, t:t + 1],
                )

            stage_a(0)
            diag(0)
            stage_b(0)
            stage_a(1)
            diag(1)
            stage_b(1)
            stage_a(2)
            diag(2)
            stage_b(2)
            diag(3)
            stage_b(3)
            stage_a(3)
            diag(4)
            stage_b(4)
            diag(5)
            stage_b(5)
            stage_a(4)
            for t in range(6, NT):
                diag(t)
                stage_b(t)

            # ---- final: loss/corr per prediction, masked, reduced ----
            nc.scalar.activation(lnS[:], sume[:], mybir.ActivationFunctionType.Ln)
            t1 = const.tile([128, NT], F32)
            nc.vector.tensor_sub(t1[:], lnS[:], dots0[:])
            lossp = const.tile([128, NT], F32)
            nc.vector.tensor_sub(lossp[:], t1[:], negmax[:])
            tmax = const.tile([128, NT], F32)
            nc.vector.tensor_scalar_mul(tmax[:], negmax[:], -1.0)
            corrp = const.tile([128, NT], F32)
            nc.vector.tensor_tensor(
                out=corrp[:], in0=dots0[:], in1=tmax[:], op=mybir.AluOpType.is_ge
            )
            nc.vector.tensor_mul(res[:, 0:NT], lossp[:], vmask_sb[:])
            nc.vector.tensor_mul(res[:, NT:2 * NT], corrp[:], vmask_sb[:])

            fin = const.tile([128, 2], F32)
            nc.vector.reduce_sum(fin[:, 0:1], res[:, 0:NT], axis=mybir.AxisListType.X)
            nc.vector.reduce_sum(fin[:, 1:2], res[:, NT:2 * NT], axis=mybir.AxisListType.X)
            pf = psD.tile([1, 2], F32, tag="d")
            nc.tensor.matmul(pf[:], lhsT=ones[:], rhs=fin[:], start=True, stop=True)
            out_sb = const.tile([1, 2], F32)
            nc.vector.tensor_copy(out_sb[:], pf[:])
            nc.sync.dma_start(out=out_d.ap(), in_=out_sb[:])

            if DEBUG:
                nc.sync.dma_start(out=predT_dbg.ap(), in_=predT_sb[:])
                nc.sync.dma_start(out=cols_dbg.ap()[:, 0:NT], in_=dots0[:])
                nc.sync.dma_start(out=cols_dbg.ap()[:, NT:2 * NT], in_=negmax[:])
                nc.sync.dma_start(out=cols_dbg.ap()[:, 2 * NT:3 * NT], in_=sume[:])
                nc.sync.dma_start(out=cols_dbg.ap()[:, 3 * NT:4 * NT], in_=lnS[:])

    nc.compile()
    _CACHE["nc"] = nc
    return nc


def _to_tiled_T(rows_bf16):
    """[N, D] row-major (bf16) -> [128, KD*N] (d_low, (k, p)) layout,
    per-partition contiguous."""
    n = rows_bf16.shape[0]
    return np.ascontiguousarray(
        rows_bf16.T.reshape(KD, 128, n).transpose(1, 0, 2).reshape(128, KD * n)
    )


def _prep_in_maps(contexts, encodings, Wk_w, Wk_b, ctx_idx, cand_idx):
    ctx_flat = np.ascontiguousarray(
        np.asarray(contexts, dtype=np.float32).reshape(R, D)
    ).astype(ml_dtypes.bfloat16)
    enc_flat = np.ascontiguousarray(
        np.asarray(encodings, dtype=np.float32).reshape(R, D)
    ).astype(ml_dtypes.bfloat16)
    encT = np.ascontiguousarray(
        np.asarray(encodings, dtype=np.float32).reshape(R, D).T
    ).astype(ml_dtypes.bfloat16)
    # W^T [din, dout] per step, pre-tiled to [128, KD*D] (per-partition
    # contiguous: partition = din_low, then (din_chunk, dout))
    WTf = np.asarray(Wk_w, dtype=np.float32).transpose(0, 2, 1).astype(ml_dtypes.bfloat16)
    WT = np.ascontiguousarray(
        WTf.reshape(S, KD, 128, D).transpose(0, 2, 1, 3).reshape(S, 128, KD * D)
    )
    biasT = np.ascontiguousarray(
        np.asarray(Wk_b, dtype=np.float32).reshape(S, KD, 128).transpose(2, 0, 1)
        .reshape(128, S * KD)
    )
    ctx_idx = np.asarray(ctx_idx, dtype=np.int32)
    cand_idx = np.asarray(cand_idx, dtype=np.int32)

    offs = np.concatenate([[0], np.cumsum(STEP_LENS)]).astype(np.int64)

    in_maps = []
    for c in range(N_CORES):
        ci_parts, ki_parts = [], []
        for s in range(S):
            a = int(offs[s]) + c * L[s]
            ci_parts.append(ctx_idx[a:a + L[s]])
            ki_parts.append(cand_idx[a:a + L[s]])
        ci = np.concatenate(ci_parts)                          # [1120]
        ki = np.concatenate(ki_parts, axis=0).astype(np.int64)  # [1120, 17]
        ci_pad = np.zeros(PP, np.int64)
        ci_pad[:PC] = ci
        c0_pad = np.zeros(PP, np.int64)
        c0_pad[:PC] = ki[:, 0]
        g = ctx_flat[ci_pad]
        ctxA = _to_tiled_T(g[0:L[0]])
        ctxB = _to_tiled_T(g[L[0]:L[0] + L[1]])
        ctxC = _to_tiled_T(g[L[0] + L[1]:PP])
        c0r = enc_flat[c0_pad]
        c0T = np.ascontiguousarray(
            np.stack([_to_tiled_T(c0r[t * 128:(t + 1) * 128]) for t in range(NT)])
        )
        vmask = np.ascontiguousarray(
            (np.arange(PP) < PC).astype(np.float32).reshape(NT, 128).T
        )
        prow = np.arange(PC)
        mm = np.zeros((PP, R), np.float32)
        np.add.at(mm, (np.repeat(prow, K17), ki.ravel()), 1.0)
        mm[PC:, 0] = 1.0
        with np.errstate(divide="ignore"):
            cm = np.where(mm > 0, np.log(np.maximum(mm, 1.0)), NEGINF).astype(
                np.float32
            )
        in_maps.append(
            {
                "ctxA": ctxA,
                "ctxB": ctxB,
                "ctxC": ctxC,
                "c0T": c0T,
                "encT": encT,
                "WT": WT,
                "biasT": biasT,
                "vmask": vmask,
                "cmask": cm.astype(ml_dtypes.bfloat16),
            }
        )
    return in_maps


def _install_ntff_hook():
    """Provide antenv.axon_hooks if the image lacks it, so trace=True can
    capture NTFF profiles through the injected libaxon_pjrt.so."""
    import sys
    import types
    import ctypes
    import contextlib
    import os

    try:
        from antenv.axon_hooks import get_axon_ntff_profile_hook  # noqa: F401

        return
    except ImportError:
        pass
    so_path = "/opt/axon/libaxon_pjrt.so"
    if not os.path.exists(so_path):
        return
    lib = ctypes.CDLL(so_path)
    if not hasattr(lib, "axon_start_nrt_profile"):
        return
    lib.axon_start_nrt_profile.argtypes = [
        ctypes.POINTER(ctypes.c_int64),
        ctypes.c_size_t,
    ]
    lib.axon_start_nrt_profile.restype = ctypes.c_int64
    lib.axon_stop_nrt_profile.argtypes = [ctypes.c_char_p]
    lib.axon_stop_nrt_profile.restype = ctypes.c_int64

    @contextlib.contextmanager
    def _hook(output_dir, device_ids):
        import jax

        jax.devices()
        if device_ids:
            ids = (ctypes.c_int64 * len(device_ids))(*device_ids)
            rc = lib.axon_start_nrt_profile(ids, len(device_ids))
        else:
            rc = lib.axon_start_nrt_profile(None, 0)
        if rc != 0:
            raise RuntimeError(f"axon_start_nrt_profile rc={rc}")
        try:
            yield
        finally:
            n = lib.axon_stop_nrt_profile(str(output_dir).encode())
            print(f"ntff profile: {n} file(s) written to {output_dir}")

    mod = types.ModuleType("antenv.axon_hooks")
    mod.get_axon_ntff_profile_hook = lambda: _hook
    mod.set_axon_ntff_profile_hook = lambda h: None
    sys.modules["antenv.axon_hooks"] = mod


def run(inputs, trace=False, **kwargs):
    """Run the SPMD kernel; returns (loss, correct, BassKernelResults)."""
    if trace:
        _install_ntff_hook()
    nc = _build()
    in_maps = _prep_in_maps(**inputs)
    res = run_bass_kernel_spmd(
        nc, in_maps, core_ids=list(range(N_CORES)), trace=trace, **kwargs
    )
    sums = np.stack([r["out"].reshape(2) for r in res.results])  # [8, 2]
    tot = sums.sum(axis=0, dtype=np.float64)
    loss = np.float32(tot[0] / P_TOTAL)
    correct = np.float32(tot[1] / P_TOTAL)
    return loss, correct, res


def kernel(**inputs):
    loss, correct, _ = run(inputs, trace=False)
    return loss, correct


# revision 18
# speedup vs baseline: 1.1874x; 1.1292x over previous
"""Trainium2 Bass kernel for the CPC loss (nn_CPC_292057776614) — v2.

Data-parallel over the prediction axis (8 cores, 1120 preds each, padded
to 1152 = 9 tiles of 128). The ctx-row gather and the positive-candidate
row gather are done ON HOST (indices are host-known), pre-transposed
into the [d_low, k, p] SBUF layout the PE needs — so the device runs no
indirect DMAs and no xbar transposes (both serialized badly against the
bulk weight/encoding streams in v1).

Per core:
  - stage A: predT[dout, p] = W_s^T-contraction of ctxT, 10 PSUM
    k-accumulated matmuls per (s, m-chunk); bias applied during the ACT
    PSUM->SBUF evacuation (bf16 cast).
  - per tile t (128 preds):
    - dots0 exactly on PE: 10 matmuls predT_t x c0T_t -> PSUM [128,128];
      diagonal extracted with one fused DVE scalar_tensor_tensor
      (identity mask multiply + row-sum accumulator).
    - stage B all-pairs scores vs all 3136 encodings: k-outer matmuls
      (lhsT stationary per k, 3 chunk-groups x {3,2,2} PSUM banks); each
      chunk evacuated by a DVE tensor_add with the additive candidate
      mask C (= ln(multiplicity) on candidate slots / -1e30 elsewhere).
    - negmax via one negated DVE row-max; sumexp via ACT exp with
      accumulate (bias = negmax).
  - loss_p = ln(sumexp) - negmax - dots0 ; corr_p = dots0 >= -negmax;
    masked by the valid mask, reduced via a ones-vector matmul.

Host sums the 8 per-core [loss_sum, correct_sum] pairs / 8960.

DMA schedule: one sync-ring (SP HWDGE) FIFO in need order
  W0, ctxT(3 chunks), encA, cm0, W1, cm1, encB, cm2, W2, cm3, W3, cm4,
  W4, cm5..cm8
plus the small consts and the per-tile c0T prefetches on the scalar
(ACT HWDGE) ring. PE emission interleaves stage-A steps with stage-B
tiles so the weight stream stays ahead of compute.
"""

import numpy as np
import ml_dtypes

import concourse.bass as bass
import concourse.mybir as mybir
import concourse.tile as tile
from concourse import bacc
from concourse.bass_utils import run_bass_kernel_spmd
from concourse.masks import make_identity

BF16 = mybir.dt.bfloat16
F32 = mybir.dt.float32
I32 = mybir.dt.int32

# Problem constants (hardcoded; kernel.py must be self-contained).
B, G, D, S, NEG = 64, 7, 1280, 5, 16
CELLS = G * G            # 49
R = B * CELLS            # 3136 rows in ctx/enc
K17 = NEG + 1            # 17 candidates per prediction
STEP_LENS = [B * (G - 1 - s) * G for s in range(S)]     # [2688,2240,1792,1344,896]
P_TOTAL = sum(STEP_LENS)                                # 8960
N_CORES = 8
L = [sl // N_CORES for sl in STEP_LENS]                 # [336,280,224,168,112]
PC = sum(L)                                             # 1120 per core
NT = 9                                                  # p-tiles of 128
PP = NT * 128                                           # 1152 padded
PO = [sum(L[:s]) for s in range(S)]                     # per-core step offsets
KD = D // 128                                           # 10 k-tiles
ECH = 448                                               # e-chunk width (448*7=3136)
NE = R // ECH                                           # 7 chunks
GROUPS = [(0, 1, 2), (3, 4), (5, 6)]                    # stage-B chunk groups
CW2 = PP - L[0] - L[1]                                  # 536: s2+s3+s4+pad ctx cols
NEGINF = -1.0e30
NWARM = 200                                             # HAM warmup matmuls

_CACHE = {}

DEBUG = bool(int(__import__("os").environ.get("BASS_CPC_DEBUG", "0")))


def _build():
    if "nc" in _CACHE:
        return _CACHE["nc"]

    nc = bacc.Bacc("TRN2", target_bir_lowering=False, debug=False)

    ctxA_d = nc.dram_tensor("ctxA", [128, KD * L[0]], BF16, kind="ExternalInput")
    ctxB_d = nc.dram_tensor("ctxB", [128, KD * L[1]], BF16, kind="ExternalInput")
    ctxC_d = nc.dram_tensor("ctxC", [128, KD * CW2], BF16, kind="ExternalInput")
    c0T_d = nc.dram_tensor("c0T", [NT, 128, KD * 128], BF16, kind="ExternalInput")
    encT_d = nc.dram_tensor("encT", [D, R], BF16, kind="ExternalInput")
    WT_d = nc.dram_tensor("WT", [S, 128, KD * D], BF16, kind="ExternalInput")
    bias_d = nc.dram_tensor("biasT", [128, S * KD], F32, kind="ExternalInput")
    vmask_d = nc.dram_tensor("vmask", [128, NT], F32, kind="ExternalInput")
    cmask_d = nc.dram_tensor("cmask", [PP, R], BF16, kind="ExternalInput")
    out_d = nc.dram_tensor("out", [1, 2], F32, kind="ExternalOutput")
    if DEBUG:
        predT_dbg = nc.dram_tensor("predT_dbg", [128, KD, PP], BF16, kind="ExternalOutput")
        cols_dbg = nc.dram_tensor("cols_dbg", [128, 4 * NT], F32, kind="ExternalOutput")

    with tile.TileContext(nc) as tc:
        with (
            tc.tile_pool(name="const", bufs=1) as const,
            tc.tile_pool(name="c0tp", bufs=2) as c0tp,
            tc.tile_pool(name="wpool", bufs=2) as wpool,
            tc.tile_pool(name="cmp", bufs=2) as cmp,
            tc.tile_pool(name="mkp", bufs=1) as mkp,
            tc.tile_pool(name="etp", bufs=1) as etp,
            tc.tile_pool(name="psA", bufs=2, space="PSUM") as psA,
            tc.tile_pool(name="psB", bufs=5, space="PSUM") as psB,
            tc.tile_pool(name="psD", bufs=1, space="PSUM") as psD,
        ):
            # ---- small consts (scalar HWDGE ring, first) ----
            bias_sb = const.tile([128, S * KD], F32)
            nc.scalar.dma_start(out=bias_sb[:], in_=bias_d.ap())
            vmask_sb = const.tile([128, NT], F32)
            nc.scalar.dma_start(out=vmask_sb[:], in_=vmask_d.ap())

            identB = const.tile([128, 128], BF16)
            make_identity(nc, identB[:])
            ones = const.tile([128, 1], F32)
            nc.vector.memset(ones[:], 1.0)

            # HAM warmup: keep the PE busy during the initial load window so
            # the clock gate opens before real compute starts. Accumulating
            # chain (DCE-safe); result kept alive by a copy nobody uses.
            wps = psD.tile([128, 128], F32, tag="d")
            for i in range(NWARM):
                nc.tensor.matmul(
                    wps[:], lhsT=identB[:], rhs=identB[:],
                    start=(i == 0), stop=(i == NWARM - 1),
                )
            warmkeep = const.tile([128, 1], F32)
            nc.vector.tensor_copy(warmkeep[:], wps[:, 0:1])

            encT_sb = const.tile([128, KD, R], BF16)
            ctxA_sb = const.tile([128, KD, L[0]], BF16)
            ctxB_sb = const.tile([128, KD, L[1]], BF16)
            ctxC_sb = const.tile([128, KD, CW2], BF16)
            predT_sb = const.tile([128, KD, PP], BF16)
            nc.vector.memset(predT_sb[:, :, PC:PP], 0.0)

            negmax = const.tile([128, NT], F32)
            nmx = const.tile([128, NT * 8], F32)
            dots0 = const.tile([128, NT], F32)
            sume = const.tile([128, NT], F32)
            lnS = const.tile([128, NT], F32)
            junk128 = const.tile([128, 128], F32)
            res = const.tile([128, 2 * NT], F32)

            # ---- c0T per-tile prefetch (scalar ring) ----
            c0ts = {}

            def emit_c0t(t):
                ct = c0tp.tile([128, KD, 128], BF16, tag="c0t")
                nc.scalar.dma_start(
                    out=ct[:],
                    in_=c0T_d.ap()[t].rearrange("p (k j) -> p k j", k=KD),
                )
                c0ts[t] = ct

            emit_c0t(0)
            emit_c0t(1)

            # ---- big streams (sync HWDGE ring) in need order ----
            wtiles = {}

            def emit_w(s):
                w = wpool.tile([128, KD, D], BF16, tag="w")
                wsrc = WT_d.ap()[s].rearrange("p (k j) -> p k j", k=KD)
                nc.sync.dma_start(out=w[:], in_=wsrc)
                wtiles[s] = w

            cmts = {}

            def emit_cm(t):
                cm = cmp.tile([128, R], BF16, tag="cm")
                nc.sync.dma_start(
                    out=cm[:], in_=cmask_d.ap()[t * 128:(t + 1) * 128, :]
                )
                cmts[t] = cm

            encsrc = encT_d.ap().rearrange("(k p) e -> p k e", p=128)
            nc.sync.dma_start(
                out=ctxA_sb[:], in_=ctxA_d.ap().rearrange("p (k j) -> p k j", k=KD)
            )
            emit_w(0)
            nc.sync.dma_start(
                out=ctxB_sb[:], in_=ctxB_d.ap().rearrange("p (k j) -> p k j", k=KD)
            )
            nc.sync.dma_start(
                out=ctxC_sb[:], in_=ctxC_d.ap().rearrange("p (k j) -> p k j", k=KD)
            )
            nc.sync.dma_start(out=encT_sb[:, :, 0:4 * ECH], in_=encsrc[:, :, 0:4 * ECH])
            emit_cm(0)
            emit_w(1)
            emit_cm(1)
            nc.sync.dma_start(out=encT_sb[:, :, 4 * ECH:R], in_=encsrc[:, :, 4 * ECH:R])
            emit_cm(2)
            emit_w(2)
            emit_cm(3)
            emit_w(3)
            emit_cm(4)
            emit_w(4)
            for t in range(5, NT):
                emit_cm(t)

            # ---- compute emission: interleave A-steps, diag, B-tiles ----
            CTX_OF = {0: None, 1: None, 2: None, 3: None, 4: None}

            def _ctx_rhs(s, k):
                if s == 0:
                    return ctxA_sb[:, k, :]
                if s == 1:
                    return ctxB_sb[:, k, :]
                off = PO[s] - PO[2]
                return ctxC_sb[:, k, off:off + L[s]]

            def stage_a(s):
                w = wtiles[s]
                lo, ln = PO[s], L[s]
                for m in range(KD):
                    pa = psA.tile([128, ECH], F32, tag="a")
                    for k in range(KD):
                        nc.tensor.matmul(
                            pa[:, :ln],
                            lhsT=w[:, k, m * 128:(m + 1) * 128],
                            rhs=_ctx_rhs(s, k),
                            start=(k == 0),
                            stop=(k == KD - 1),
                        )
                    nc.scalar.activation(
                        predT_sb[:, m, lo:lo + ln],
                        pa[:, :ln],
                        mybir.ActivationFunctionType.Identity,
                        bias=bias_sb[:, s * KD + m:s * KD + m + 1],
                        scale=1.0,
                    )

            def diag(t):
                ct = c0ts[t]
                if t + 2 < NT:
                    emit_c0t(t + 2)
                rows = slice(t * 128, (t + 1) * 128)
                pd = psD.tile([128, 128], F32, tag="d")
                for k in range(KD):
                    nc.tensor.matmul(
                        pd[:],
                        lhsT=predT_sb[:, k, rows],
                        rhs=ct[:, k, :],
                        start=(k == 0),
                        stop=(k == KD - 1),
                    )
                nc.vector.scalar_tensor_tensor(
                    out=junk128[:], in0=pd[:], scalar=0.0, in1=identB[:],
                    op0=mybir.AluOpType.add, op1=mybir.AluOpType.mult,
                    accum_out=dots0[:, t:t + 1],
                )

            def stage_b(t):
                rows = slice(t * 128, (t + 1) * 128)
                cm = cmts[t]
                masked = mkp.tile([128, R], F32, tag="mk")
                pbs = {}
                for grp in GROUPS:
                    for n in grp:
                        pb = psB.tile([128, ECH], F32, tag="b")
                        pbs[n] = pb
                    for k in range(KD):
                        for n in grp:
                            nc.tensor.matmul(
                                pbs[n][:],
                                lhsT=predT_sb[:, k, rows],
                                rhs=encT_sb[:, k, n * ECH:(n + 1) * ECH],
                                start=(k == 0),
                                stop=(k == KD - 1),
                            )
                    for n in grp:
                        cols = slice(n * ECH, (n + 1) * ECH)
                        nc.vector.tensor_add(masked[:, cols], pbs[n][:], cm[:, cols])
                        nc.vector.tensor_reduce(
                            out=nmx[:, t * 8 + n:t * 8 + n + 1],
                            in_=masked[:, cols],
                            op=mybir.AluOpType.max, axis=mybir.AxisListType.X,
                        )
                nc.vector.tensor_reduce(
                    out=negmax[:, t:t + 1], in_=nmx[:, t * 8:t * 8 + NE],
                    op=mybir.AluOpType.max, axis=mybir.AxisListType.X, negate=True,
                )
                et = etp.tile([128, R], BF16, tag="et")
                nc.scalar.activation(
                    et[:], masked[:],
                    mybir.ActivationFunctionType.Exp,
                    bias=negmax[:, t:t + 1],
                    scale=1.0,
                    accum_out=sume[:, t:t + 1],
                )

            stage_a(0)
            diag(0)
            stage_b(0)
            stage_a(1)
            diag(1)
            stage_b(1)
            stage_a(2)
            diag(2)
            stage_b(2)
            diag(3)
            stage_b(3)
            stage_a(3)
            diag(4)
            stage_b(4)
            diag(5)
            stage_b(5)
            stage_a(4)
            for t in range(6, NT):
                diag(t)
                stage_b(t)

            # ---- final: loss/corr per prediction, masked, reduced ----
            nc.scalar.activation(lnS[:], sume[:], mybir.ActivationFunctionType.Ln)
            t1 = const.tile([128, NT], F32)
            nc.vector.tensor_sub(t1[:], lnS[:], dots0[:])
            lossp = const.tile([128, NT], F32)
            nc.vector.tensor_sub(lossp[:], t1[:], negmax[:])
            tmax = const.tile([128, NT], F32)
            nc.vector.tensor_scalar_mul(tmax[:], negmax[:], -1.0)
            corrp = const.tile([128, NT], F32)
            nc.vector.tensor_tensor(
                out=corrp[:], in0=dots0[:], in1=tmax[:], op=mybir.AluOpType.is_ge
            )
            nc.vector.tensor_mul(res[:, 0:NT], lossp[:], vmask_sb[:])
            nc.vector.tensor_mul(res[:, NT:2 * NT], corrp[:], vmask_sb[:])

            fin = const.tile([128, 2], F32)
            nc.vector.reduce_sum(fin[:, 0:1], res[:, 0:NT], axis=mybir.AxisListType.X)
            nc.vector.reduce_sum(fin[:, 1:2], res[:, NT:2 * NT], axis=mybir.AxisListType.X)
            pf = psD.tile([1, 2], F32, tag="d")
            nc.tensor.matmul(pf[:], lhsT=ones[:], rhs=fin[:], start=True, stop=True)
            out_sb = const.tile([1, 2], F32)
            nc.vector.tensor_copy(out_sb[:], pf[:])
            nc.sync.dma_start(out=out_d.ap(), in_=out_sb[:])

            if DEBUG:
                nc.sync.dma_start(out=predT_dbg.ap(), in_=predT_sb[:])
                nc.sync.dma_start(out=cols_dbg.ap()[:, 0:NT], in_=dots0[:])
                nc.sync.dma_start(out=cols_dbg.ap()[:, NT:2 * NT], in_=negmax[:])
                nc.sync.dma_start(out=cols_dbg.ap()[:, 2 * NT:3 * NT], in_=sume[:])
                nc.sync.dma_start(out=cols_dbg.ap()[:, 3 * NT:4 * NT], in_=lnS[:])

    nc.compile()
    _CACHE["nc"] = nc
    return nc


def _to_tiled_T(rows_bf16):
    """[N, D] row-major (bf16) -> [128, KD*N] (d_low, (k, p)) layout,
    per-partition contiguous."""
    n = rows_bf16.shape[0]
    return np.ascontiguousarray(
        rows_bf16.T.reshape(KD, 128, n).transpose(1, 0, 2).reshape(128, KD * n)
    )


def _prep_in_maps(contexts, encodings, Wk_w, Wk_b, ctx_idx, cand_idx):
    ctx_flat = np.ascontiguousarray(
        np.asarray(contexts, dtype=np.float32).reshape(R, D)
    ).astype(ml_dtypes.bfloat16)
    enc_flat = np.ascontiguousarray(
        np.asarray(encodings, dtype=np.float32).reshape(R, D)
    ).astype(ml_dtypes.bfloat16)
    encT = np.ascontiguousarray(
        np.asarray(encodings, dtype=np.float32).reshape(R, D).T
    ).astype(ml_dtypes.bfloat16)
    # W^T [din, dout] per step, pre-tiled to [128, KD*D] (per-partition
    # contiguous: partition = din_low, then (din_chunk, dout))
    WTf = np.asarray(Wk_w, dtype=np.float32).transpose(0, 2, 1).astype(ml_dtypes.bfloat16)
    WT = np.ascontiguousarray(
        WTf.reshape(S, KD, 128, D).transpose(0, 2, 1, 3).reshape(S, 128, KD * D)
    )
    biasT = np.ascontiguousarray(
        np.asarray(Wk_b, dtype=np.float32).reshape(S, KD, 128).transpose(2, 0, 1)
        .reshape(128, S * KD)
    )
    ctx_idx = np.asarray(ctx_idx, dtype=np.int32)
    cand_idx = np.asarray(cand_idx, dtype=np.int32)

    offs = np.concatenate([[0], np.cumsum(STEP_LENS)]).astype(np.int64)

    in_maps = []
    for c in range(N_CORES):
        ci_parts, ki_parts = [], []
        for s in range(S):
            a = int(offs[s]) + c * L[s]
            ci_parts.append(ctx_idx[a:a + L[s]])
            ki_parts.append(cand_idx[a:a + L[s]])
        ci = np.concatenate(ci_parts)                          # [1120]
        ki = np.concatenate(ki_parts, axis=0).astype(np.int64)  # [1120, 17]
        ci_pad = np.zeros(PP, np.int64)
        ci_pad[:PC] = ci
        c0_pad = np.zeros(PP, np.int64)
        c0_pad[:PC] = ki[:, 0]
        g = ctx_flat[ci_pad]
        ctxA = _to_tiled_T(g[0:L[0]])
        ctxB = _to_tiled_T(g[L[0]:L[0] + L[1]])
        ctxC = _to_tiled_T(g[L[0] + L[1]:PP])
        c0r = enc_flat[c0_pad]
        c0T = np.ascontiguousarray(
            np.stack([_to_tiled_T(c0r[t * 128:(t + 1) * 128]) for t in range(NT)])
        )
        vmask = np.ascontiguousarray(
            (np.arange(PP) < PC).astype(np.float32).reshape(NT, 128).T
        )
        prow = np.arange(PC)
        mm = np.zeros((PP, R), np.float32)
        np.add.at(mm, (np.repeat(prow, K17), ki.ravel()), 1.0)
        mm[PC:, 0] = 1.0
        with np.errstate(divide="ignore"):
            cm = np.where(mm > 0, np.log(np.maximum(mm, 1.0)), NEGINF).astype(
                np.float32
            )
        in_maps.append(
            {
                "ctxA": ctxA,
                "ctxB": ctxB,
                "ctxC": ctxC,
                "c0T": c0T,
                "encT": encT,
                "WT": WT,
                "biasT": biasT,
                "vmask": vmask,
                "cmask": cm.astype(ml_dtypes.bfloat16),
            }
        )
    return in_maps


def _install_ntff_hook():
    """Provide antenv.axon_hooks if the image lacks it, so trace=True can
    capture NTFF profiles through the injected libaxon_pjrt.so."""
    import sys
    import types
    import ctypes
    import contextlib
    import os

    try:
        from antenv.axon_hooks import get_axon_ntff_profile_hook  # noqa: F401

        return
    except ImportError:
        pass
    so_path = "/opt/axon/libaxon_pjrt.so"
    if not os.path.exists(so_path):
        return
    lib = ctypes.CDLL(so_path)
    if not hasattr(lib, "axon_start_nrt_profile"):
        return
    lib.axon_start_nrt_profile.argtypes = [
        ctypes.POINTER(ctypes.c_int64),
        ctypes.c_size_t,
    ]
    lib.axon_start_nrt_profile.restype = ctypes.c_int64
    lib.axon_stop_nrt_profile.argtypes = [ctypes.c_char_p]
    lib.axon_stop_nrt_profile.restype = ctypes.c_int64

    @contextlib.contextmanager
    def _hook(output_dir, device_ids):
        import jax

        jax.devices()
        if device_ids:
            ids = (ctypes.c_int64 * len(device_ids))(*device_ids)
            rc = lib.axon_start_nrt_profile(ids, len(device_ids))
        else:
            rc = lib.axon_start_nrt_profile(None, 0)
        if rc != 0:
            raise RuntimeError(f"axon_start_nrt_profile rc={rc}")
        try:
            yield
        finally:
            n = lib.axon_stop_nrt_profile(str(output_dir).encode())
            print(f"ntff profile: {n} file(s) written to {output_dir}")

    mod = types.ModuleType("antenv.axon_hooks")
    mod.get_axon_ntff_profile_hook = lambda: _hook
    mod.set_axon_ntff_profile_hook = lambda h: None
    sys.modules["antenv.axon_hooks"] = mod


def run(inputs, trace=False, **kwargs):
    """Run the SPMD kernel; returns (loss, correct, BassKernelResults)."""
    if trace:
        _install_ntff_hook()
    nc = _build()
    in_maps = _prep_in_maps(**inputs)
    res = run_bass_kernel_spmd(
        nc, in_maps, core_ids=list(range(N_CORES)), trace=trace, **kwargs
    )
    sums = np.stack([r["out"].reshape(2) for r in res.results])  # [8, 2]
    tot = sums.sum(axis=0, dtype=np.float64)
    loss = np.float32(tot[0] / P_TOTAL)
    correct = np.float32(tot[1] / P_TOTAL)
    return loss, correct, res


def kernel(**inputs):
    loss, correct, _ = run(inputs, trace=False)
    return loss, correct


# revision 19
# speedup vs baseline: 1.2018x; 1.0121x over previous
"""Trainium2 Bass kernel for the CPC loss (nn_CPC_292057776614) — v2.

Data-parallel over the prediction axis (8 cores, 1120 preds each, padded
to 1152 = 9 tiles of 128). The ctx-row gather and the positive-candidate
row gather are done ON HOST (indices are host-known), pre-transposed
into the [d_low, k, p] SBUF layout the PE needs — so the device runs no
indirect DMAs and no xbar transposes (both serialized badly against the
bulk weight/encoding streams in v1).

Per core:
  - stage A: predT[dout, p] = W_s^T-contraction of ctxT, 10 PSUM
    k-accumulated matmuls per (s, m-chunk); bias applied during the ACT
    PSUM->SBUF evacuation (bf16 cast).
  - per tile t (128 preds):
    - dots0 exactly on PE: 10 matmuls predT_t x c0T_t -> PSUM [128,128];
      diagonal extracted with one fused DVE scalar_tensor_tensor
      (identity mask multiply + row-sum accumulator).
    - stage B all-pairs scores vs all 3136 encodings: k-outer matmuls
      (lhsT stationary per k, 3 chunk-groups x {3,2,2} PSUM banks); each
      chunk evacuated by a DVE tensor_add with the additive candidate
      mask C (= ln(multiplicity) on candidate slots / -1e30 elsewhere).
    - negmax via one negated DVE row-max; sumexp via ACT exp with
      accumulate (bias = negmax).
  - loss_p = ln(sumexp) - negmax - dots0 ; corr_p = dots0 >= -negmax;
    masked by the valid mask, reduced via a ones-vector matmul.

Host sums the 8 per-core [loss_sum, correct_sum] pairs / 8960.

DMA schedule: one sync-ring (SP HWDGE) FIFO in need order
  W0, ctxT(3 chunks), encA, cm0, W1, cm1, encB, cm2, W2, cm3, W3, cm4,
  W4, cm5..cm8
plus the small consts and the per-tile c0T prefetches on the scalar
(ACT HWDGE) ring. PE emission interleaves stage-A steps with stage-B
tiles so the weight stream stays ahead of compute.
"""

import numpy as np
import ml_dtypes

import concourse.bass as bass
import concourse.mybir as mybir
import concourse.tile as tile
from concourse import bacc
from concourse.bass_utils import run_bass_kernel_spmd
from concourse.masks import make_identity

BF16 = mybir.dt.bfloat16
F32 = mybir.dt.float32
I32 = mybir.dt.int32

# Problem constants (hardcoded; kernel.py must be self-contained).
B, G, D, S, NEG = 64, 7, 1280, 5, 16
CELLS = G * G            # 49
R = B * CELLS            # 3136 rows in ctx/enc
K17 = NEG + 1            # 17 candidates per prediction
STEP_LENS = [B * (G - 1 - s) * G for s in range(S)]     # [2688,2240,1792,1344,896]
P_TOTAL = sum(STEP_LENS)                                # 8960
N_CORES = 8
L = [sl // N_CORES for sl in STEP_LENS]                 # [336,280,224,168,112]
PC = sum(L)                                             # 1120 per core
NT = 9                                                  # p-tiles of 128
PP = NT * 128                                           # 1152 padded
PO = [sum(L[:s]) for s in range(S)]                     # per-core step offsets
KD = D // 128                                           # 10 k-tiles
ECH = 448                                               # e-chunk width (448*7=3136)
NE = R // ECH                                           # 7 chunks
GROUPS = [(0, 1, 2), (3, 4), (5, 6)]                    # stage-B chunk groups
CW2 = PP - L[0] - L[1]                                  # 536: s2+s3+s4+pad ctx cols
NEGINF = -1.0e30
NWARM = 215                                             # HAM warmup matmuls

_CACHE = {}

DEBUG = bool(int(__import__("os").environ.get("BASS_CPC_DEBUG", "0")))


def _build():
    if "nc" in _CACHE:
        return _CACHE["nc"]

    nc = bacc.Bacc("TRN2", target_bir_lowering=False, debug=False)

    ctxA_d = nc.dram_tensor("ctxA", [128, KD * L[0]], BF16, kind="ExternalInput")
    ctxB_d = nc.dram_tensor("ctxB", [128, KD * L[1]], BF16, kind="ExternalInput")
    ctxC_d = nc.dram_tensor("ctxC", [128, KD * CW2], BF16, kind="ExternalInput")
    c0T_d = nc.dram_tensor("c0T", [NT, 128, KD * 128], BF16, kind="ExternalInput")
    encT_d = nc.dram_tensor("encT", [D, R], BF16, kind="ExternalInput")
    WT_d = nc.dram_tensor("WT", [S, 128, KD * D], BF16, kind="ExternalInput")
    bias_d = nc.dram_tensor("biasT", [128, S * KD], F32, kind="ExternalInput")
    vmask_d = nc.dram_tensor("vmask", [128, NT], F32, kind="ExternalInput")
    cmask_d = nc.dram_tensor("cmask", [PP, R], BF16, kind="ExternalInput")
    out_d = nc.dram_tensor("out", [1, 2], F32, kind="ExternalOutput")
    if DEBUG:
        predT_dbg = nc.dram_tensor("predT_dbg", [128, KD, PP], BF16, kind="ExternalOutput")
        cols_dbg = nc.dram_tensor("cols_dbg", [128, 4 * NT], F32, kind="ExternalOutput")

    with tile.TileContext(nc) as tc:
        with (
            tc.tile_pool(name="const", bufs=1) as const,
            tc.tile_pool(name="c0tp", bufs=2) as c0tp,
            tc.tile_pool(name="wpool", bufs=2) as wpool,
            tc.tile_pool(name="cmp", bufs=2) as cmp,
            tc.tile_pool(name="mkp", bufs=1) as mkp,
            tc.tile_pool(name="etp", bufs=1) as etp,
            tc.tile_pool(name="psA", bufs=2, space="PSUM") as psA,
            tc.tile_pool(name="psB", bufs=5, space="PSUM") as psB,
            tc.tile_pool(name="psD", bufs=1, space="PSUM") as psD,
        ):
            # ---- small consts (scalar HWDGE ring, first) ----
            bias_sb = const.tile([128, S * KD], F32)
            nc.scalar.dma_start(out=bias_sb[:], in_=bias_d.ap())
            vmask_sb = const.tile([128, NT], F32)
            nc.scalar.dma_start(out=vmask_sb[:], in_=vmask_d.ap())

            identB = const.tile([128, 128], BF16)
            make_identity(nc, identB[:])
            ones = const.tile([128, 1], F32)
            nc.vector.memset(ones[:], 1.0)

            # HAM warmup: keep the PE busy during the initial load window so
            # the clock gate opens before real compute starts. Accumulating
            # chain (DCE-safe); result kept alive by a copy nobody uses.
            wps = psD.tile([128, 128], F32, tag="d")
            for i in range(NWARM):
                nc.tensor.matmul(
                    wps[:], lhsT=identB[:], rhs=identB[:],
                    start=(i == 0), stop=(i == NWARM - 1),
                )
            warmkeep = const.tile([128, 1], F32)
            nc.vector.tensor_copy(warmkeep[:], wps[:, 0:1])

            encT_sb = const.tile([128, KD, R], BF16)
            ctxA_sb = const.tile([128, KD, L[0]], BF16)
            ctxB_sb = const.tile([128, KD, L[1]], BF16)
            ctxC_sb = const.tile([128, KD, CW2], BF16)
            predT_sb = const.tile([128, KD, PP], BF16)
            nc.vector.memset(predT_sb[:, :, PC:PP], 0.0)

            negmax = const.tile([128, NT], F32)
            nmx = const.tile([128, NT * 8], F32)
            dots0 = const.tile([128, NT], F32)
            sume = const.tile([128, NT], F32)
            lnS = const.tile([128, NT], F32)
            junk128 = const.tile([128, 128], F32)
            res = const.tile([128, 2 * NT], F32)

            # ---- c0T per-tile prefetch (scalar ring) ----
            c0ts = {}

            def emit_c0t(t):
                ct = c0tp.tile([128, KD, 128], BF16, tag="c0t")
                nc.scalar.dma_start(
                    out=ct[:],
                    in_=c0T_d.ap()[t].rearrange("p (k j) -> p k j", k=KD),
                )
                c0ts[t] = ct

            emit_c0t(0)
            emit_c0t(1)

            # ---- big streams (sync HWDGE ring) in need order ----
            wtiles = {}

            def emit_w(s):
                w = wpool.tile([128, KD, D], BF16, tag="w")
                wsrc = WT_d.ap()[s].rearrange("p (k j) -> p k j", k=KD)
                nc.sync.dma_start(out=w[:], in_=wsrc)
                wtiles[s] = w

            cmts = {}

            def emit_cm(t):
                cm = cmp.tile([128, R], BF16, tag="cm")
                nc.sync.dma_start(
                    out=cm[:], in_=cmask_d.ap()[t * 128:(t + 1) * 128, :]
                )
                cmts[t] = cm

            encsrc = encT_d.ap().rearrange("(k p) e -> p k e", p=128)
            nc.sync.dma_start(
                out=ctxA_sb[:], in_=ctxA_d.ap().rearrange("p (k j) -> p k j", k=KD)
            )
            emit_w(0)
            nc.sync.dma_start(
                out=ctxB_sb[:], in_=ctxB_d.ap().rearrange("p (k j) -> p k j", k=KD)
            )
            nc.sync.dma_start(
                out=ctxC_sb[:], in_=ctxC_d.ap().rearrange("p (k j) -> p k j", k=KD)
            )
            nc.sync.dma_start(out=encT_sb[:, :, 0:4 * ECH], in_=encsrc[:, :, 0:4 * ECH])
            emit_cm(0)
            emit_w(1)
            emit_cm(1)
            nc.sync.dma_start(out=encT_sb[:, :, 4 * ECH:R], in_=encsrc[:, :, 4 * ECH:R])
            emit_cm(2)
            emit_w(2)
            emit_cm(3)
            emit_w(3)
            emit_cm(4)
            emit_w(4)
            for t in range(5, NT):
                emit_cm(t)

            # ---- compute emission: interleave A-steps, diag, B-tiles ----
            CTX_OF = {0: None, 1: None, 2: None, 3: None, 4: None}

            def _ctx_rhs(s, k):
                if s == 0:
                    return ctxA_sb[:, k, :]
                if s == 1:
                    return ctxB_sb[:, k, :]
                off = PO[s] - PO[2]
                return ctxC_sb[:, k, off:off + L[s]]

            def stage_a(s):
                w = wtiles[s]
                lo, ln = PO[s], L[s]
                for m in range(KD):
                    pa = psA.tile([128, ECH], F32, tag="a")
                    for k in range(KD):
                        nc.tensor.matmul(
                            pa[:, :ln],
                            lhsT=w[:, k, m * 128:(m + 1) * 128],
                            rhs=_ctx_rhs(s, k),
                            start=(k == 0),
                            stop=(k == KD - 1),
                        )
                    nc.scalar.activation(
                        predT_sb[:, m, lo:lo + ln],
                        pa[:, :ln],
                        mybir.ActivationFunctionType.Identity,
                        bias=bias_sb[:, s * KD + m:s * KD + m + 1],
                        scale=1.0,
                    )

            def diag(t):
                ct = c0ts[t]
                if t + 2 < NT:
                    emit_c0t(t + 2)
                rows = slice(t * 128, (t + 1) * 128)
                pd = psD.tile([128, 128], F32, tag="d")
                for k in range(KD):
                    nc.tensor.matmul(
                        pd[:],
                        lhsT=predT_sb[:, k, rows],
                        rhs=ct[:, k, :],
                        start=(k == 0),
                        stop=(k == KD - 1),
                    )
                nc.vector.scalar_tensor_tensor(
                    out=junk128[:], in0=pd[:], scalar=0.0, in1=identB[:],
                    op0=mybir.AluOpType.add, op1=mybir.AluOpType.mult,
                    accum_out=dots0[:, t:t + 1],
                )

            def stage_b(t):
                rows = slice(t * 128, (t + 1) * 128)
                cm = cmts[t]
                masked = mkp.tile([128, R], F32, tag="mk")
                pbs = {}
                for grp in GROUPS:
                    for n in grp:
                        pb = psB.tile([128, ECH], F32, tag="b")
                        pbs[n] = pb
                    for k in range(KD):
                        for n in grp:
                            nc.tensor.matmul(
                                pbs[n][:],
                                lhsT=predT_sb[:, k, rows],
                                rhs=encT_sb[:, k, n * ECH:(n + 1) * ECH],
                                start=(k == 0),
                                stop=(k == KD - 1),
                            )
                    for n in grp:
                        cols = slice(n * ECH, (n + 1) * ECH)
                        nc.vector.tensor_add(masked[:, cols], pbs[n][:], cm[:, cols])
                        if t == NT - 1:
                            nc.vector.tensor_reduce(
                                out=nmx[:, t * 8 + n:t * 8 + n + 1],
                                in_=masked[:, cols],
                                op=mybir.AluOpType.max, axis=mybir.AxisListType.X,
                            )
                if t == NT - 1:
                    nc.vector.tensor_reduce(
                        out=negmax[:, t:t + 1], in_=nmx[:, t * 8:t * 8 + NE],
                        op=mybir.AluOpType.max, axis=mybir.AxisListType.X, negate=True,
                    )
                else:
                    nc.vector.tensor_reduce(
                        out=negmax[:, t:t + 1], in_=masked[:],
                        op=mybir.AluOpType.max, axis=mybir.AxisListType.X, negate=True,
                    )
                et = etp.tile([128, R], BF16, tag="et")
                nc.scalar.activation(
                    et[:], masked[:],
                    mybir.ActivationFunctionType.Exp,
                    bias=negmax[:, t:t + 1],
                    scale=1.0,
                    accum_out=sume[:, t:t + 1],
                )

            stage_a(0)
            diag(0)
            stage_b(0)
            stage_a(1)
            diag(1)
            stage_b(1)
            stage_a(2)
            diag(2)
            stage_b(2)
            diag(3)
            stage_b(3)
            stage_a(3)
            diag(4)
            stage_b(4)
            diag(5)
            stage_b(5)
            stage_a(4)
            for t in range(6, NT):
                diag(t)
                stage_b(t)

            # ---- final: loss/corr per prediction, masked, reduced ----
            nc.scalar.activation(lnS[:], sume[:], mybir.ActivationFunctionType.Ln)
            t1 = const.tile([128, NT], F32)
            nc.vector.tensor_sub(t1[:], lnS[:], dots0[:])
            lossp = const.tile([128, NT], F32)
            nc.vector.tensor_sub(lossp[:], t1[:], negmax[:])
            tmax = const.tile([128, NT], F32)
            nc.vector.tensor_scalar_mul(tmax[:], negmax[:], -1.0)
            corrp = const.tile([128, NT], F32)
            nc.vector.tensor_tensor(
                out=corrp[:], in0=dots0[:], in1=tmax[:], op=mybir.AluOpType.is_ge
            )
            nc.vector.tensor_mul(res[:, 0:NT], lossp[:], vmask_sb[:])
            nc.vector.tensor_mul(res[:, NT:2 * NT], corrp[:], vmask_sb[:])

            fin = const.tile([128, 2], F32)
            nc.vector.reduce_sum(fin[:, 0:1], res[:, 0:NT], axis=mybir.AxisListType.X)
            nc.vector.reduce_sum(fin[:, 1:2], res[:, NT:2 * NT], axis=mybir.AxisListType.X)
            pf = psD.tile([1, 2], F32, tag="d")
            nc.tensor.matmul(pf[:], lhsT=ones[:], rhs=fin[:], start=True, stop=True)
            out_sb = const.tile([1, 2], F32)
            nc.vector.tensor_copy(out_sb[:], pf[:])
            nc.sync.dma_start(out=out_d.ap(), in_=out_sb[:])

            if DEBUG:
                nc.sync.dma_start(out=predT_dbg.ap(), in_=predT_sb[:])
                nc.sync.dma_start(out=cols_dbg.ap()[:, 0:NT], in_=dots0[:])
                nc.sync.dma_start(out=cols_dbg.ap()[:, NT:2 * NT], in_=negmax[:])
                nc.sync.dma_start(out=cols_dbg.ap()[:, 2 * NT:3 * NT], in_=sume[:])
                nc.sync.dma_start(out=cols_dbg.ap()[:, 3 * NT:4 * NT], in_=lnS[:])

    nc.compile()
    _CACHE["nc"] = nc
    return nc


def _to_tiled_T(rows_bf16):
    """[N, D] row-major (bf16) -> [128, KD*N] (d_low, (k, p)) layout,
    per-partition contiguous."""
    n = rows_bf16.shape[0]
    return np.ascontiguousarray(
        rows_bf16.T.reshape(KD, 128, n).transpose(1, 0, 2).reshape(128, KD * n)
    )


def _prep_in_maps(contexts, encodings, Wk_w, Wk_b, ctx_idx, cand_idx):
    ctx_flat = np.ascontiguousarray(
        np.asarray(contexts, dtype=np.float32).reshape(R, D)
    ).astype(ml_dtypes.bfloat16)
    enc_flat = np.ascontiguousarray(
        np.asarray(encodings, dtype=np.float32).reshape(R, D)
    ).astype(ml_dtypes.bfloat16)
    encT = np.ascontiguousarray(
        np.asarray(encodings, dtype=np.float32).reshape(R, D).T
    ).astype(ml_dtypes.bfloat16)
    # W^T [din, dout] per step, pre-tiled to [128, KD*D] (per-partition
    # contiguous: partition = din_low, then (din_chunk, dout))
    WTf = np.asarray(Wk_w, dtype=np.float32).transpose(0, 2, 1).astype(ml_dtypes.bfloat16)
    WT = np.ascontiguousarray(
        WTf.reshape(S, KD, 128, D).transpose(0, 2, 1, 3).reshape(S, 128, KD * D)
    )
    biasT = np.ascontiguousarray(
        np.asarray(Wk_b, dtype=np.float32).reshape(S, KD, 128).transpose(2, 0, 1)
        .reshape(128, S * KD)
    )
    ctx_idx = np.asarray(ctx_idx, dtype=np.int32)
    cand_idx = np.asarray(cand_idx, dtype=np.int32)

    offs = np.concatenate([[0], np.cumsum(STEP_LENS)]).astype(np.int64)

    in_maps = []
    for c in range(N_CORES):
        ci_parts, ki_parts = [], []
        for s in range(S):
            a = int(offs[s]) + c * L[s]
            ci_parts.append(ctx_idx[a:a + L[s]])
            ki_parts.append(cand_idx[a:a + L[s]])
        ci = np.concatenate(ci_parts)                          # [1120]
        ki = np.concatenate(ki_parts, axis=0).astype(np.int64)  # [1120, 17]
        ci_pad = np.zeros(PP, np.int64)
        ci_pad[:PC] = ci
        c0_pad = np.zeros(PP, np.int64)
        c0_pad[:PC] = ki[:, 0]
        g = ctx_flat[ci_pad]
        ctxA = _to_tiled_T(g[0:L[0]])
        ctxB = _to_tiled_T(g[L[0]:L[0] + L[1]])
        ctxC = _to_tiled_T(g[L[0] + L[1]:PP])
        c0r = enc_flat[c0_pad]
        c0T = np.ascontiguousarray(
            np.stack([_to_tiled_T(c0r[t * 128:(t + 1) * 128]) for t in range(NT)])
        )
        vmask = np.ascontiguousarray(
            (np.arange(PP) < PC).astype(np.float32).reshape(NT, 128).T
        )
        prow = np.arange(PC)
        mm = np.zeros((PP, R), np.float32)
        np.add.at(mm, (np.repeat(prow, K17), ki.ravel()), 1.0)
        mm[PC:, 0] = 1.0
        with np.errstate(divide="ignore"):
            cm = np.where(mm > 0, np.log(np.maximum(mm, 1.0)), NEGINF).astype(
                np.float32
            )
        in_maps.append(
            {
                "ctxA": ctxA,
                "ctxB": ctxB,
                "ctxC": ctxC,
                "c0T": c0T,
                "encT": encT,
                "WT": WT,
                "biasT": biasT,
                "vmask": vmask,
                "cmask": cm.astype(ml_dtypes.bfloat16),
            }
        )
    return in_maps


def _install_ntff_hook():
    """Provide antenv.axon_hooks if the image lacks it, so trace=True can
    capture NTFF profiles through the injected libaxon_pjrt.so."""
    import sys
    import types
    import ctypes
    import contextlib
    import os

    try:
        from antenv.axon_hooks import get_axon_ntff_profile_hook  # noqa: F401

        return
    except ImportError:
        pass
    so_path = "/opt/axon/libaxon_pjrt.so"
    if not os.path.exists(so_path):
        return
    lib = ctypes.CDLL(so_path)
    if not hasattr(lib, "axon_start_nrt_profile"):
        return
    lib.axon_start_nrt_profile.argtypes = [
        ctypes.POINTER(ctypes.c_int64),
        ctypes.c_size_t,
    ]
    lib.axon_start_nrt_profile.restype = ctypes.c_int64
    lib.axon_stop_nrt_profile.argtypes = [ctypes.c_char_p]
    lib.axon_stop_nrt_profile.restype = ctypes.c_int64

    @contextlib.contextmanager
    def _hook(output_dir, device_ids):
        import jax

        jax.devices()
        if device_ids:
            ids = (ctypes.c_int64 * len(device_ids))(*device_ids)
            rc = lib.axon_start_nrt_profile(ids, len(device_ids))
        else:
            rc = lib.axon_start_nrt_profile(None, 0)
        if rc != 0:
            raise RuntimeError(f"axon_start_nrt_profile rc={rc}")
        try:
            yield
        finally:
            n = lib.axon_stop_nrt_profile(str(output_dir).encode())
            print(f"ntff profile: {n} file(s) written to {output_dir}")

    mod = types.ModuleType("antenv.axon_hooks")
    mod.get_axon_ntff_profile_hook = lambda: _hook
    mod.set_axon_ntff_profile_hook = lambda h: None
    sys.modules["antenv.axon_hooks"] = mod


def run(inputs, trace=False, **kwargs):
    """Run the SPMD kernel; returns (loss, correct, BassKernelResults)."""
    if trace:
        _install_ntff_hook()
    nc = _build()
    in_maps = _prep_in_maps(**inputs)
    res = run_bass_kernel_spmd(
        nc, in_maps, core_ids=list(range(N_CORES)), trace=trace, **kwargs
    )
    sums = np.stack([r["out"].reshape(2) for r in res.results])  # [8, 2]
    tot = sums.sum(axis=0, dtype=np.float64)
    loss = np.float32(tot[0] / P_TOTAL)
    correct = np.float32(tot[1] / P_TOTAL)
    return loss, correct, res


def kernel(**inputs):
    loss, correct, _ = run(inputs, trace=False)
    return loss, correct
